# revision 3
# baseline (speedup 1.0000x reference)
"""Causal multi-head attention with RoPE for Trainium2, sharded over 8 NeuronCores.

Problem: B=4, T=2048, C=768, H=12, D=64, fp32 in/out.
    q,k,v = x @ wq/wk/wv  (per-head reshape), RoPE(q,k), causal softmax(q k^T/sqrt(D)) v,
    out = concat_heads @ wo.

Sharding: core c -> (batch b = c//2, head-group g = c%2 covering heads g*6..g*6+5).
Each core computes its 6 heads' attention and a partial output projection
y_c = out_heads(g) @ wo[rows g]; the host sums the two partials per batch.

On-core dataflow (all matmul operands bf16, fp32 PSUM accumulation):
  - host passes x^T (bf16) so every matmul contracts along partitions.
  - q^T,k^T in [head_dim, T] layout; RoPE via a block-rotation matmul +
    cos/sin tensor ops, software-pipelined so the rotation matmuls of
    block i-1 run while block i's projection accumulates (no PE stalls
    on the PSUM->SBUF copy).
  - scores transposed: S^T[k, q] = k^T.T @ q^T with K=64 row-pairing
    (even head at partitions 0:64, odd at 64:128 -> concurrent row groups).
  - P = exp(S/8) on ScalarE -> bf16.  ScalarE is the serial bottleneck
    (~13.4M exps/core at 1 elem/cycle/lane), so phase 2 starts with the
    (p=0, qc=3) score block zipped against the v-projection: exp begins
    as soon as phase 1 drains and runs back-to-back to the end.
  - PV with a ones-row appended to V: out_unnorm^T[d, q] and l[q] in one
    accumulated matmul chain; pt tiles are buffered 8-deep per head so
    the deferred PV never blocks the exp stream.
  - softmax normalization: l row -> partition 0, gpsimd.partition_broadcast,
    reciprocal_approx_fast, TT multiply.
  - output projection accumulates 3 head-pair chunks into [128, 768] PSUM.
"""

import numpy as np
from contextlib import ExitStack

B, T, C, H, D = 4, 2048, 768, 12, 64
HPC = 6          # heads per core
NP = 3           # head-pair tiles per core
CC = C // 128    # 6 contraction chunks
TT = T // 128    # 16 t tiles
QC = T // 512    # 4 q chunks
KC = T // 128    # 16 k chunks

_COMPILED = None


def _rope_tables():
    import ml_dtypes
    inv_freq = 1.0 / (10000.0 ** (np.arange(0, D, 2, dtype=np.float64) / D))  # [32]
    t = np.arange(T, dtype=np.float64)
    freqs = np.outer(t, inv_freq)                      # [T, 32]
    cosT = np.cos(freqs).T.astype(np.float32)          # [32, T]
    sinT = np.sin(freqs).T.astype(np.float32)
    ccat = np.tile(cosT, (4, 1)).astype(ml_dtypes.bfloat16)   # [128, T]
    scat = np.tile(sinT, (4, 1)).astype(ml_dtypes.bfloat16)
    return np.ascontiguousarray(ccat), np.ascontiguousarray(scat)


def _rot_matrix():
    import ml_dtypes
    # rotate_half as a matmul: rot = R @ q (q in [D, T] layout), per 64-row block
    R = np.zeros((D, D), dtype=np.float32)
    R[0:32, 32:64] = -np.eye(32, dtype=np.float32)
    R[32:64, 0:32] = np.eye(32, dtype=np.float32)
    R2 = np.zeros((128, 128), dtype=np.float32)
    R2[0:64, 0:64] = R
    R2[64:128, 64:128] = R
    return np.ascontiguousarray(R2.T.astype(ml_dtypes.bfloat16))  # lhsT for out = R2 @ q


def _build_program():
    import concourse.tile as tile
    from concourse import bacc, mybir

    F32 = mybir.dt.float32
    BF16 = mybir.dt.bfloat16
    EXP = mybir.ActivationFunctionType.Exp

    nc = bacc.Bacc("TRN2", target_bir_lowering=False, debug=False, num_devices=8)

    xT_d = nc.dram_tensor("xT", [C, T], BF16, kind="ExternalInput").ap()
    wq_d = nc.dram_tensor("wq", [C, HPC * D], BF16, kind="ExternalInput").ap()
    wk_d = nc.dram_tensor("wk", [C, HPC * D], BF16, kind="ExternalInput").ap()
    wv_d = nc.dram_tensor("wv", [C, HPC * D], BF16, kind="ExternalInput").ap()
    wo_d = nc.dram_tensor("wo", [HPC * D, C], BF16, kind="ExternalInput").ap()
    ccat_d = nc.dram_tensor("ccat", [128, T], BF16, kind="ExternalInput").ap()
    scat_d = nc.dram_tensor("scat", [128, T], BF16, kind="ExternalInput").ap()
    r2t_d = nc.dram_tensor("r2t", [128, 128], BF16, kind="ExternalInput").ap()
    utri_d = nc.dram_tensor("utri", [128, 128], BF16, kind="ExternalInput").ap()
    eband_d = nc.dram_tensor("eband", [128, 128], BF16, kind="ExternalInput").ap()
    y_d = nc.dram_tensor("y", [T, C], F32, kind="ExternalOutput").ap()

    with tile.TileContext(nc) as tc, ExitStack() as ctx:
        big_pool = ctx.enter_context(tc.tile_pool(name="big", bufs=1))
        q_all = big_pool.tile([128, NP, T], BF16)
        k_all = big_pool.tile([128, NP, T], BF16)
        v_aug = big_pool.tile([128, KC, HPC, D + 1], BF16)
        out_norm = big_pool.tile([128, NP, T], BF16)

        # ---- phase 0: input DMAs, critical-path first on each ring ----
        cst_pool = ctx.enter_context(tc.tile_pool(name="cst", bufs=1))
        xt_pool = ctx.enter_context(tc.tile_pool(name="xt", bufs=1))
        xt_sb = xt_pool.tile([128, CC, T], BF16)

        r2t = cst_pool.tile([128, 128], BF16)
        nc.sync.dma_start(r2t[:], r2t_d)
        wv_sb = cst_pool.tile([128, CC, HPC * D], BF16)
        wo_sb = cst_pool.tile([128, NP, C], BF16)
        utri = cst_pool.tile([128, 128], BF16)
        eband = cst_pool.tile([128, 128], BF16)
        exp_warm = cst_pool.tile([1, 2], F32)

        nc.gpsimd.memset(v_aug[:, :, :, D:D + 1], 1.0)

        with tc.tile_pool(name="w", bufs=1) as w_pool, \
             tc.tile_pool(name="const", bufs=1) as const_pool, \
             tc.tile_pool(name="p1ps", bufs=4, space="PSUM") as p1ps, \
             tc.tile_pool(name="p1tmp", bufs=2) as p1tmp:
            wq_sb = w_pool.tile([128, CC, HPC * D], BF16)
            nc.sync.dma_start(wq_sb[:], wq_d.rearrange("(cc p) d -> p cc d", p=128))
            xT_r = xT_d.rearrange("(cc p) t -> p cc t", p=128)
            for cc in range(CC):
                nc.sync.dma_start(xt_sb[:, cc, :], xT_r[:, cc, :])
            wk_sb = w_pool.tile([128, CC, HPC * D], BF16)
            nc.sync.dma_start(wk_sb[:], wk_d.rearrange("(cc p) d -> p cc d", p=128))

            ccat = const_pool.tile([128, T], BF16)
            nc.scalar.dma_start(ccat[:], ccat_d)
            scat = const_pool.tile([128, T], BF16)
            nc.scalar.dma_start(scat[:], scat_d)
            nc.scalar.dma_start(utri[:], utri_d)
            nc.scalar.dma_start(eband[:], eband_d)
            nc.scalar.dma_start(wv_sb[:], wv_d.rearrange("(cc p) d -> p cc d", p=128))
            nc.scalar.dma_start(wo_sb[:], wo_d.rearrange("(hc p) c -> p hc c", p=128))

            # HAM warmup + Exp table preload while the input DMAs land
            warm_t = p1ps.tile([128, 1024], F32, tag="p1")
            warm = warm_t[:, 0:128]
            nc.scalar.activation(exp_warm[:], r2t[0:1, 0:2], EXP)
            for _ in range(40):
                nc.tensor.matmul(warm[:], r2t[:], r2t[:], start=True, stop=True)

            # ---- phase 1: q^T, k^T projections + RoPE, software-pipelined ----
            def finish_block(blk):
                dt, dst, qraw = blk
                ps_rot = [None, None]
                for hh in range(2):
                    ps_r = p1ps.tile([128, 1024], F32, tag="p1", name="ps_r")
                    for tq in range(2):
                        nc.tensor.matmul(
                            ps_r[:, tq * 512:(tq + 1) * 512],
                            r2t[:],
                            qraw[:, hh * 1024 + tq * 512:
                                  hh * 1024 + (tq + 1) * 512],
                            start=True, stop=True,
                        )
                    ps_rot[hh] = ps_r
                nc.vector.tensor_mul(dst[:, dt, :], qraw[:], ccat[:])
                for hh in range(2):
                    hsl = slice(hh * 1024, (hh + 1) * 1024)
                    nc.vector.tensor_mul(qraw[:, hsl], ps_rot[hh][:, :],
                                         scat[:, hsl])
                nc.vector.tensor_add(dst[:, dt, :], dst[:, dt, :], qraw[:])

            prev = None
            for dt in range(NP):
                for w_sb, dst in ((wq_sb, q_all), (wk_sb, k_all)):
                    qraw = p1tmp.tile([128, T], BF16, tag="qraw")
                    for hh in range(2):
                        hsl = slice(hh * 1024, (hh + 1) * 1024)
                        ps_q = p1ps.tile([128, 1024], F32, tag="p1", name="ps_q")
                        for cc in range(CC):
                            for tq in range(2):
                                nc.tensor.matmul(
                                    ps_q[:, tq * 512:(tq + 1) * 512],
                                    w_sb[:, cc, dt * 128:(dt + 1) * 128],
                                    xt_sb[:, cc,
                                          hh * 1024 + tq * 512:
                                          hh * 1024 + (tq + 1) * 512],
                                    start=(cc == 0), stop=(cc == CC - 1),
                                )
                        nc.scalar.copy(qraw[:, hsl], ps_q[:, :])
                    if prev is not None:
                        finish_block(prev)
                    prev = (dt, dst, qraw)
            finish_block(prev)

        # ---- phase 2: fused v-projection + attention + output projection ----
        # PSUM (8 banks): s0,s1 [128,1024] (4) + aux rotation (4): ps_v/pv/y
        with tc.tile_pool(name="s_ps", bufs=1, space="PSUM") as s_psp, \
             tc.tile_pool(name="aux_ps", bufs=4, space="PSUM") as aux_psp, \
             tc.tile_pool(name="p_sb", bufs=8) as p_sbp, \
             tc.tile_pool(name="l_sb", bufs=2) as l_sbp, \
             tc.tile_pool(name="r_sb", bufs=2) as r_sbp, \
             tc.tile_pool(name="y_sb", bufs=2) as y_sbp:

            def emit_vproj(tt):
                ps_v = aux_psp.tile([128, HPC * D], F32, tag="aux", name="ps_v")
                for cc in range(CC):
                    nc.tensor.matmul(
                        ps_v[:, 0:HPC * D],
                        xt_sb[:, cc, tt * 128:(tt + 1) * 128],
                        wv_sb[:, cc, :],
                        start=(cc == 0), stop=(cc == CC - 1),
                    )
                nc.vector.tensor_copy(
                    v_aug[:, tt, :, 0:D],
                    ps_v[:, 0:HPC * D].rearrange("p (h d) -> p h d", d=D),
                )

            def emit_scores_offdiag(p, qc, kcs):
                """One group: 1-2 kc blocks x 2 heads; returns exp'd pt tiles."""
                s_t = [s_psp.tile([128, 1024], F32, tag=f"s{h01}",
                                  name=f"s_t{h01}") for h01 in (0, 1)]
                for j, kc in enumerate(kcs):
                    for h01 in (0, 1):
                        r0, r1 = h01 * 64, h01 * 64 + 64
                        nc.tensor.matmul(
                            s_t[h01][:, j * 512:(j + 1) * 512],
                            k_all[r0:r1, p, kc * 128:(kc + 1) * 128],
                            q_all[r0:r1, p, qc * 512:(qc + 1) * 512],
                            start=True, stop=True,
                        )
                pts = []
                for h01 in (0, 1):
                    pt = p_sbp.tile([128, 1024], BF16, tag=f"pt{h01}")
                    w = len(kcs) * 512
                    nc.scalar.activation(pt[:, 0:w], s_t[h01][:, 0:w], EXP,
                                         scale=0.125)
                    pts.append(pt)
                return pts

            def emit_pv_offdiag(p, qc, kcs, pts, pv):
                for j, kc in enumerate(kcs):
                    for h01 in (0, 1):
                        nc.tensor.matmul(
                            pv[h01][:],
                            v_aug[:, kc, p * 2 + h01, :],
                            pts[h01][:, j * 512:(j + 1) * 512],
                            start=(kc == 0), stop=False,
                        )

            # diagonal tiles: half 0 = j0(512)+j1(384), half 1 = j2(256)+j3(128)
            DIAG_SEGS = (((0, 0, 512), (1, 512, 384)),
                         ((2, 0, 256), (3, 256, 128)))

            def emit_scores_diag(p, qc, segs):
                s_d = [s_psp.tile([128, 1024], F32, tag=f"s{h01}",
                                  name=f"s_d{h01}") for h01 in (0, 1)]
                for j, off, wj in segs:
                    kc = 4 * qc + j
                    for h01 in (0, 1):
                        r0, r1 = h01 * 64, h01 * 64 + 64
                        nc.tensor.matmul(
                            s_d[h01][:, off:off + wj],
                            k_all[r0:r1, p, kc * 128:(kc + 1) * 128],
                            q_all[r0:r1, p, qc * 512 + 128 * j:qc * 512 + 512],
                            start=True, stop=False,
                        )
                    for h01 in (0, 1):
                        nc.tensor.matmul(
                            s_d[h01][:, off:off + 128],
                            utri[:], eband[:],
                            start=False, stop=True,
                        )
                pts = []
                for h01 in (0, 1):
                    pt_d = p_sbp.tile([128, 1024], BF16, tag=f"pt{h01}",
                                      name="pt_d")
                    wtot = sum(sg[2] for sg in segs)
                    nc.scalar.activation(pt_d[:, 0:wtot], s_d[h01][:, 0:wtot],
                                         EXP, scale=0.125)
                    pts.append(pt_d)
                return pts

            def emit_pv_diag(p, qc, segs, pts, pv, last):
                for j, off, wj in segs:
                    kc = 4 * qc + j
                    for h01 in (0, 1):
                        nc.tensor.matmul(
                            pv[h01][:, 128 * j:512],
                            v_aug[:, kc, p * 2 + h01, :],
                            pts[h01][:, off:off + wj],
                            start=(kc == 0), stop=(last and j == 3),
                        )

            def emit_norm(p, qc, pv):
                for h01 in (0, 1):
                    lrow = l_sbp.tile([1, 512], F32, tag=f"l{h01}")
                    nc.vector.tensor_copy(lrow[0:1, :], pv[h01][64:65, :])
                    rbc = r_sbp.tile([64, 512], F32, tag=f"r{h01}")
                    nc.gpsimd.partition_broadcast(rbc[:], lrow[0:1, :],
                                                  channels=64)
                    nc.vector.reciprocal_approx_fast(rbc[:], rbc[:])
                    nc.vector.tensor_mul(
                        out_norm[h01 * 64:h01 * 64 + 64, p,
                                 qc * 512:(qc + 1) * 512],
                        pv[h01][0:64, :],
                        rbc[:],
                    )

            def emit_outproj(qc):
                for tt in range(4 * qc, 4 * qc + 4):
                    y_a = aux_psp.tile([128, 512], F32, tag="aux", name="y_a")
                    y_b = aux_psp.tile([128, 256], F32, tag="aux", name="y_b")
                    for hc in range(NP):
                        lhsT = out_norm[:, hc, tt * 128:(tt + 1) * 128]
                        nc.tensor.matmul(y_a[:, 0:512], lhsT,
                                         wo_sb[:, hc, 0:512],
                                         start=(hc == 0), stop=(hc == NP - 1))
                        nc.tensor.matmul(y_b[:, 0:256], lhsT,
                                         wo_sb[:, hc, 512:768],
                                         start=(hc == 0), stop=(hc == NP - 1))
                    yt = y_sbp.tile([128, C], F32, tag="yt")
                    nc.vector.tensor_copy(yt[:, 0:512], y_a[:, 0:512])
                    nc.vector.tensor_copy(yt[:, 512:768], y_b[:, 0:256])
                    nc.sync.dma_start(y_d[tt * 128:(tt + 1) * 128, :], yt[:])

            def emit_attn(p, qc):
                """Standard ping-pong: scores -> exp -> PV per group."""
                pv = [aux_psp.tile([65, 512], F32, tag="aux", name=f"pv{h01}")
                      for h01 in (0, 1)]
                for g0 in range(0, 4 * qc, 2):
                    kcs = list(range(g0, min(g0 + 2, 4 * qc)))
                    pts = emit_scores_offdiag(p, qc, kcs)
                    emit_pv_offdiag(p, qc, kcs, pts, pv)
                for half, segs in enumerate(DIAG_SEGS):
                    pts = emit_scores_diag(p, qc, segs)
                    emit_pv_diag(p, qc, segs, pts, pv, last=(half == 1))
                emit_norm(p, qc, pv)

            # --- (p0, qc3): scores+exp zipped against vproj, PV deferred ---
            qc = 3
            held = []
            vprog = 0
            for g0 in range(0, 4 * qc, 2):
                kcs = list(range(g0, g0 + 2))
                held.append(("off", kcs, emit_scores_offdiag(0, qc, kcs)))
                while vprog < (g0 // 2 + 1) * 3 and vprog < KC:
                    emit_vproj(vprog)
                    vprog += 1
            for segs in DIAG_SEGS:
                held.append(("diag", segs, emit_scores_diag(0, qc, segs)))
            while vprog < KC:
                emit_vproj(vprog)
                vprog += 1
            pv = [aux_psp.tile([65, 512], F32, tag="aux", name=f"pv{h01}")
                  for h01 in (0, 1)]
            for kind, arg, pts in held:
                if kind == "off":
                    emit_pv_offdiag(0, qc, arg, pts, pv)
                else:
                    emit_pv_diag(0, qc, arg, pts, pv,
                                 last=(arg is DIAG_SEGS[1]))
            emit_norm(0, qc, pv)

            emit_attn(1, 3)
            emit_attn(2, 3)
            emit_outproj(3)
            for qc in (2, 1, 0):
                for p in range(NP):
                    emit_attn(p, qc)
                emit_outproj(qc)

    nc.compile()
    return nc


# make mybir importable inside _build_program's nested scopes
from concourse import mybir  # noqa: E402


def _get_compiled():
    global _COMPILED
    if _COMPILED is None:
        _COMPILED = _build_program()
    return _COMPILED


def _make_in_maps(inputs):
    import ml_dtypes

    BF = ml_dtypes.bfloat16
    x = np.asarray(inputs["x"], dtype=np.float32)
    wq = np.asarray(inputs["wq"], dtype=np.float32).astype(BF)
    wk = np.asarray(inputs["wk"], dtype=np.float32).astype(BF)
    wv = np.asarray(inputs["wv"], dtype=np.float32).astype(BF)
    wo = np.asarray(inputs["wo"], dtype=np.float32).astype(BF)

    ccat, scat = _rope_tables()
    r2t = _rot_matrix()
    m = np.arange(128)
    utri = (m[:, None] <= m[None, :]).astype(BF)
    eband = np.zeros((128, 128), dtype=np.float32)
    eband[np.arange(1, 128), np.arange(127)] = -1e9
    eband = eband.astype(BF)

    xTs = [np.ascontiguousarray(x[b].T.astype(BF)) for b in range(B)]
    in_maps = []
    for c in range(8):
        b, g = c // 2, c % 2
        sl = slice(g * HPC * D, (g + 1) * HPC * D)
        in_maps.append(dict(
            xT=xTs[b],
            wq=np.ascontiguousarray(wq[:, sl]),
            wk=np.ascontiguousarray(wk[:, sl]),
            wv=np.ascontiguousarray(wv[:, sl]),
            wo=np.ascontiguousarray(wo[sl, :]),
            ccat=ccat, scat=scat, r2t=r2t, utri=utri, eband=eband,
        ))
    return in_maps


def kernel(x, wq, wk, wv, wo, mask):
    """Full inputs in, full output out. Shards across 8 NeuronCores internally.

    The mask input is the standard causal mask produced by setup_inputs();
    causality is implemented directly on-device.
    """
    from concourse.bass_utils import run_bass_kernel_spmd

    in_maps = _make_in_maps(dict(x=x, wq=wq, wk=wk, wv=wv, wo=wo))

    nc = _get_compiled()
    res = run_bass_kernel_spmd(nc, in_maps, list(range(8)))
    out = np.empty((B, T, C), dtype=np.float32)
    for b in range(B):
        out[b] = res.results[2 * b]["y"] + res.results[2 * b + 1]["y"]
    return out


# revision 6
# speedup vs baseline: 1.0594x; 1.0594x over previous
"""Causal multi-head attention with RoPE for Trainium2, sharded over 8 NeuronCores.

Problem: B=4, T=2048, C=768, H=12, D=64, fp32 in/out.
    q,k,v = x @ wq/wk/wv  (per-head reshape), RoPE(q,k), causal softmax(q k^T/sqrt(D)) v,
    out = concat_heads @ wo.

Sharding: core c -> (batch b = c//2, head-group g = c%2 covering heads g*6..g*6+5).
Each core computes its 6 heads' attention and a partial output projection
y_c = out_heads(g) @ wo[rows g]; the host sums the two partials per batch.

On-core dataflow (bf16 matmul operands, fp32 PSUM accumulation).  The two
serial resources are the TensorE matmul stream and the ScalarE exp stream
(~13.4M exps/core at 1 elem/cycle/lane), so the kernel is organised to keep
both saturated from early on:
  - phase 1 (q/k projection + RoPE) holds only 4 PSUM banks (the rotation
    matmuls recycle the projection banks) so the score/softmax pipeline
    (4 banks) coexists: the (p=0, qc=3) score blocks + exps zip INTO the
    tail projection blocks, starting the exp stream ~30us in.
  - RoPE chain per block: PSUM->bf16 copies on DVE, rotation matmul on PE,
    sin-muls on DVE into a separate tile (no WAR serialization), cos-mul
    and the final add on the otherwise idle GPSIMD.
  - scores transposed: S^T[k, q] = k^T.T @ q^T with K=64 row-pairing
    (even head at partitions 0:64, odd at 64:128 -> concurrent row groups).
  - after phase 1: the (p=1, qc=3) scores zip against the v-projection,
    then the deferred PV chains (pt tiles buffered 16-deep per head) run,
    then standard scores->exp->PV ping-pong for the rest.
  - PV appends a ones-row to V: out_unnorm^T[d, q] and l[q] in one
    accumulated matmul chain per (head, q-chunk).
  - normalization: l row -> partition 0, gpsimd.partition_broadcast,
    reciprocal_approx_fast, TT multiply.
"""

import numpy as np
from contextlib import ExitStack

B, T, C, H, D = 4, 2048, 768, 12, 64
HPC = 6          # heads per core
NP = 3           # head-pair tiles per core
CC = C // 128    # 6 contraction chunks
TT = T // 128    # 16 t tiles
QC = T // 512    # 4 q chunks
KC = T // 128    # 16 k chunks

_COMPILED = None


def _rope_tables():
    import ml_dtypes
    inv_freq = 1.0 / (10000.0 ** (np.arange(0, D, 2, dtype=np.float64) / D))  # [32]
    t = np.arange(T, dtype=np.float64)
    freqs = np.outer(t, inv_freq)                      # [T, 32]
    cosT = np.cos(freqs).T.astype(np.float32)          # [32, T]
    sinT = np.sin(freqs).T.astype(np.float32)
    ccat = np.tile(cosT, (4, 1)).astype(ml_dtypes.bfloat16)   # [128, T]
    scat = np.tile(sinT, (4, 1)).astype(ml_dtypes.bfloat16)
    return np.ascontiguousarray(ccat), np.ascontiguousarray(scat)


def _rot_matrix():
    import ml_dtypes
    # rotate_half as a matmul: rot = R @ q (q in [D, T] layout), per 64-row block
    R = np.zeros((D, D), dtype=np.float32)
    R[0:32, 32:64] = -np.eye(32, dtype=np.float32)
    R[32:64, 0:32] = np.eye(32, dtype=np.float32)
    R2 = np.zeros((128, 128), dtype=np.float32)
    R2[0:64, 0:64] = R
    R2[64:128, 64:128] = R
    return np.ascontiguousarray(R2.T.astype(ml_dtypes.bfloat16))  # lhsT for out = R2 @ q


def _build_program():
    import concourse.tile as tile
    from concourse import bacc, mybir

    F32 = mybir.dt.float32
    BF16 = mybir.dt.bfloat16
    EXP = mybir.ActivationFunctionType.Exp

    nc = bacc.Bacc("TRN2", target_bir_lowering=False, debug=False, num_devices=8)

    xT_d = nc.dram_tensor("xT", [C, T], BF16, kind="ExternalInput").ap()
    wq_d = nc.dram_tensor("wq", [C, HPC * D], BF16, kind="ExternalInput").ap()
    wk_d = nc.dram_tensor("wk", [C, HPC * D], BF16, kind="ExternalInput").ap()
    wv_d = nc.dram_tensor("wv", [C, HPC * D], BF16, kind="ExternalInput").ap()
    wo_d = nc.dram_tensor("wo", [HPC * D, C], BF16, kind="ExternalInput").ap()
    ccat_d = nc.dram_tensor("ccat", [128, T], BF16, kind="ExternalInput").ap()
    scat_d = nc.dram_tensor("scat", [128, T], BF16, kind="ExternalInput").ap()
    r2t_d = nc.dram_tensor("r2t", [128, 128], BF16, kind="ExternalInput").ap()
    utri_d = nc.dram_tensor("utri", [128, 128], BF16, kind="ExternalInput").ap()
    eband_d = nc.dram_tensor("eband", [128, 128], BF16, kind="ExternalInput").ap()
    y_d = nc.dram_tensor("y", [T, C], F32, kind="ExternalOutput").ap()

    with tile.TileContext(nc) as tc, ExitStack() as ctx:
        big_pool = ctx.enter_context(tc.tile_pool(name="big", bufs=1))
        q_all = big_pool.tile([128, NP, T], BF16)
        k_all = big_pool.tile([128, NP, T], BF16)
        v_aug = big_pool.tile([128, KC, HPC, D + 1], BF16)
        out_norm = big_pool.tile([128, NP, T], BF16)

        cst_pool = ctx.enter_context(tc.tile_pool(name="cst", bufs=1))
        xt_pool = ctx.enter_context(tc.tile_pool(name="xt", bufs=1))
        xt_sb = xt_pool.tile([128, CC, T], BF16)

        # score/softmax PSUM banks (4) — alive for the whole kernel
        s_psp = ctx.enter_context(tc.tile_pool(name="s_ps", bufs=1, space="PSUM"))
        # pt tiles buffered 16 deep per head: the (p0/p1, qc3) PV chains are
        # deferred past phase 1 / vproj without ever blocking the exp stream
        p_sbp = ctx.enter_context(tc.tile_pool(name="p_sb", bufs=16))
        l_sbp = ctx.enter_context(tc.tile_pool(name="l_sb", bufs=2))
        r_sbp = ctx.enter_context(tc.tile_pool(name="r_sb", bufs=2))
        y_sbp = ctx.enter_context(tc.tile_pool(name="y_sb", bufs=2))

        # ---- phase 0: input DMAs, critical-path first on each ring ----
        r2t = cst_pool.tile([128, 128], BF16)
        nc.sync.dma_start(r2t[:], r2t_d)
        wv_sb = cst_pool.tile([128, CC, HPC * D], BF16)
        wo_sb = cst_pool.tile([128, NP, C], BF16)
        utri = cst_pool.tile([128, 128], BF16)
        eband = cst_pool.tile([128, 128], BF16)
        exp_warm = cst_pool.tile([1, 2], F32)

        nc.gpsimd.memset(v_aug[:, :, :, D:D + 1], 1.0)

        # ---------- attention building blocks ----------
        def emit_scores_offdiag(p, qc, kcs):
            """One group: 1-2 kc blocks x 2 heads; returns exp'd pt tiles."""
            s_t = [s_psp.tile([128, 1024], F32, tag=f"s{h01}",
                              name=f"s_t{h01}") for h01 in (0, 1)]
            for j, kc in enumerate(kcs):
                for h01 in (0, 1):
                    r0, r1 = h01 * 64, h01 * 64 + 64
                    nc.tensor.matmul(
                        s_t[h01][:, j * 512:(j + 1) * 512],
                        k_all[r0:r1, p, kc * 128:(kc + 1) * 128],
                        q_all[r0:r1, p, qc * 512:(qc + 1) * 512],
                        start=True, stop=True,
                    )
            pts = []
            for h01 in (0, 1):
                pt = p_sbp.tile([128, 1024], BF16, tag=f"pt{h01}")
                w = len(kcs) * 512
                nc.scalar.activation(pt[:, 0:w], s_t[h01][:, 0:w], EXP,
                                     scale=0.125)
                pts.append(pt)
            return pts

        def emit_pv_offdiag(p, qc, kcs, pts, pv):
            for j, kc in enumerate(kcs):
                for h01 in (0, 1):
                    nc.tensor.matmul(
                        pv[h01][:],
                        v_aug[:, kc, p * 2 + h01, :],
                        pts[h01][:, j * 512:(j + 1) * 512],
                        start=(kc == 0), stop=False,
                    )

        # diagonal tiles: half 0 = j0(512)+j1(384), half 1 = j2(256)+j3(128)
        DIAG_SEGS = (((0, 0, 512), (1, 512, 384)),
                     ((2, 0, 256), (3, 256, 128)))

        def emit_scores_diag(p, qc, segs):
            s_d = [s_psp.tile([128, 1024], F32, tag=f"s{h01}",
                              name=f"s_d{h01}") for h01 in (0, 1)]
            for j, off, wj in segs:
                kc = 4 * qc + j
                for h01 in (0, 1):
                    r0, r1 = h01 * 64, h01 * 64 + 64
                    nc.tensor.matmul(
                        s_d[h01][:, off:off + wj],
                        k_all[r0:r1, p, kc * 128:(kc + 1) * 128],
                        q_all[r0:r1, p, qc * 512 + 128 * j:qc * 512 + 512],
                        start=True, stop=False,
                    )
                for h01 in (0, 1):
                    nc.tensor.matmul(
                        s_d[h01][:, off:off + 128],
                        utri[:], eband[:],
                        start=False, stop=True,
                    )
            pts = []
            for h01 in (0, 1):
                pt_d = p_sbp.tile([128, 1024], BF16, tag=f"pt{h01}",
                                  name="pt_d")
                wtot = sum(sg[2] for sg in segs)
                nc.scalar.activation(pt_d[:, 0:wtot], s_d[h01][:, 0:wtot],
                                     EXP, scale=0.125)
                pts.append(pt_d)
            return pts

        def emit_pv_diag(p, qc, segs, pts, pv, last):
            for j, off, wj in segs:
                kc = 4 * qc + j
                for h01 in (0, 1):
                    nc.tensor.matmul(
                        pv[h01][:, 128 * j:512],
                        v_aug[:, kc, p * 2 + h01, :],
                        pts[h01][:, off:off + wj],
                        start=(kc == 0), stop=(last and j == 3),
                    )

        def emit_norm(p, qc, pv):
            for h01 in (0, 1):
                lrow = l_sbp.tile([1, 512], F32, tag=f"l{h01}")
                nc.vector.tensor_copy(lrow[0:1, :], pv[h01][64:65, :])
                rbc = r_sbp.tile([64, 512], F32, tag=f"r{h01}")
                nc.gpsimd.partition_broadcast(rbc[:], lrow[0:1, :],
                                              channels=64)
                nc.vector.reciprocal_approx_fast(rbc[:], rbc[:])
                nc.vector.tensor_mul(
                    out_norm[h01 * 64:h01 * 64 + 64, p,
                             qc * 512:(qc + 1) * 512],
                    pv[h01][0:64, :],
                    rbc[:],
                )

        def score_group_closures(p, qc):
            groups = []
            for g0 in range(0, 4 * qc, 2):
                kcs = list(range(g0, min(g0 + 2, 4 * qc)))
                groups.append(("off", kcs,
                               (lambda kk: lambda: emit_scores_offdiag(p, qc, kk))(kcs)))
            for segs in DIAG_SEGS:
                groups.append(("diag", segs,
                               (lambda ss: lambda: emit_scores_diag(p, qc, ss))(segs)))
            return groups

        # ---- phase 1 + zipped (p0, qc3) scores ----
        with tc.tile_pool(name="w", bufs=1) as w_pool, \
             tc.tile_pool(name="const", bufs=1) as const_pool, \
             tc.tile_pool(name="p1ps", bufs=2, space="PSUM") as p1ps, \
             tc.tile_pool(name="p1tmp", bufs=2) as p1tmp:
            wq_sb = w_pool.tile([128, CC, HPC * D], BF16)
            nc.sync.dma_start(wq_sb[:], wq_d.rearrange("(cc p) d -> p cc d", p=128))
            xT_r = xT_d.rearrange("(cc p) t -> p cc t", p=128)
            for cc in range(CC):
                nc.sync.dma_start(xt_sb[:, cc, :], xT_r[:, cc, :])
            wk_sb = w_pool.tile([128, CC, HPC * D], BF16)
            nc.sync.dma_start(wk_sb[:], wk_d.rearrange("(cc p) d -> p cc d", p=128))

            ccat = const_pool.tile([128, T], BF16)
            nc.scalar.dma_start(ccat[:], ccat_d)
            scat = const_pool.tile([128, T], BF16)
            nc.scalar.dma_start(scat[:], scat_d)
            nc.scalar.dma_start(utri[:], utri_d)
            nc.scalar.dma_start(eband[:], eband_d)
            nc.scalar.dma_start(wv_sb[:], wv_d.rearrange("(cc p) d -> p cc d", p=128))
            nc.scalar.dma_start(wo_sb[:], wo_d.rearrange("(hc p) c -> p hc c", p=128))

            # HAM warmup + Exp table preload while the input DMAs land
            warm_t = p1ps.tile([128, 1024], F32, tag="p1")
            warm = warm_t[:, 0:128]
            nc.scalar.activation(exp_warm[:], r2t[0:1, 0:2], EXP)
            for _ in range(24):
                nc.tensor.matmul(warm[:], r2t[:], r2t[:], start=True, stop=True)

            def finish_block(blk):
                """Rotation matmuls + RoPE combine for a finished block."""
                dt, dst, qraw, sin_t = blk
                # cos path on gpsimd (SBUF-only), independent of the sin path
                nc.gpsimd.tensor_mul(dst[:, dt, :], qraw[:], ccat[:])
                for hh in range(2):
                    hsl = slice(hh * 1024, (hh + 1) * 1024)
                    ps_r = p1ps.tile([128, 1024], F32, tag="p1", name="ps_r")
                    for tq in range(2):
                        nc.tensor.matmul(
                            ps_r[:, tq * 512:(tq + 1) * 512],
                            r2t[:],
                            qraw[:, hh * 1024 + tq * 512:
                                  hh * 1024 + (tq + 1) * 512],
                            start=True, stop=True,
                        )
                    nc.vector.tensor_mul(sin_t[:, hsl], ps_r[:, :], scat[:, hsl])
                nc.gpsimd.tensor_add(dst[:, dt, :], dst[:, dt, :], sin_t[:])

            held0 = []          # (p0, qc3) score groups, exp'd during phase 1
            zgroups = score_group_closures(0, 3)
            zi = 0

            prev = None
            for dt in range(NP):
                for w_sb, dst in ((wq_sb, q_all), (wk_sb, k_all)):
                    i = dt * 2 + (0 if w_sb is wq_sb else 1)
                    qraw = p1tmp.tile([128, T], BF16, tag="qraw")
                    sin_t = p1tmp.tile([128, T], BF16, tag="sin")
                    for hh in range(2):
                        hsl = slice(hh * 1024, (hh + 1) * 1024)
                        ps_q = p1ps.tile([128, 1024], F32, tag="p1", name="ps_q")
                        for cc in range(CC):
                            for tq in range(2):
                                nc.tensor.matmul(
                                    ps_q[:, tq * 512:(tq + 1) * 512],
                                    w_sb[:, cc, dt * 128:(dt + 1) * 128],
                                    xt_sb[:, cc,
                                          hh * 1024 + tq * 512:
                                          hh * 1024 + (tq + 1) * 512],
                                    start=(cc == 0), stop=(cc == CC - 1),
                                )
                        nc.vector.tensor_copy(qraw[:, hsl], ps_q[:, :])
                    if prev is not None:
                        finish_block(prev)
                    prev = (dt, dst, qraw, sin_t)
                    if i >= 3:  # q/k for p0 are ready from block 2 onwards
                        for _ in range(2):
                            if zi < len(zgroups):
                                kind, arg, fn = zgroups[zi]
                                held0.append((kind, arg, fn()))
                                zi += 1
            finish_block(prev)
            while zi < len(zgroups):
                kind, arg, fn = zgroups[zi]
                held0.append((kind, arg, fn()))
                zi += 1

        # ---- phase 2: v-projection zipped with (p1,qc3) scores, then
        #      deferred PVs, then standard ping-pong attention ----
        with tc.tile_pool(name="aux_ps", bufs=4, space="PSUM") as aux_psp:

            def emit_vproj(tt):
                ps_v = aux_psp.tile([128, HPC * D], F32, tag="aux", name="ps_v")
                for cc in range(CC):
                    nc.tensor.matmul(
                        ps_v[:, 0:HPC * D],
                        xt_sb[:, cc, tt * 128:(tt + 1) * 128],
                        wv_sb[:, cc, :],
                        start=(cc == 0), stop=(cc == CC - 1),
                    )
                nc.vector.tensor_copy(
                    v_aug[:, tt, :, 0:D],
                    ps_v[:, 0:HPC * D].rearrange("p (h d) -> p h d", d=D),
                )

            def emit_outproj(qc):
                for tt in range(4 * qc, 4 * qc + 4):
                    y_a = aux_psp.tile([128, 512], F32, tag="aux", name="y_a")
                    y_b = aux_psp.tile([128, 256], F32, tag="aux", name="y_b")
                    for hc in range(NP):
                        lhsT = out_norm[:, hc, tt * 128:(tt + 1) * 128]
                        nc.tensor.matmul(y_a[:, 0:512], lhsT,
                                         wo_sb[:, hc, 0:512],
                                         start=(hc == 0), stop=(hc == NP - 1))
                        nc.tensor.matmul(y_b[:, 0:256], lhsT,
                                         wo_sb[:, hc, 512:768],
                                         start=(hc == 0), stop=(hc == NP - 1))
                    yt = y_sbp.tile([128, C], F32, tag="yt")
                    nc.vector.tensor_copy(yt[:, 0:512], y_a[:, 0:512])
                    nc.vector.tensor_copy(yt[:, 512:768], y_b[:, 0:256])
                    nc.sync.dma_start(y_d[tt * 128:(tt + 1) * 128, :], yt[:])

            def emit_pv_all(p, qc, held):
                pv = [aux_psp.tile([65, 512], F32, tag="aux", name=f"pv{h01}")
                      for h01 in (0, 1)]
                for kind, arg, pts in held:
                    if kind == "off":
                        emit_pv_offdiag(p, qc, arg, pts, pv)
                    else:
                        emit_pv_diag(p, qc, arg, pts, pv,
                                     last=(arg is DIAG_SEGS[1]))
                emit_norm(p, qc, pv)

            def emit_attn(p, qc):
                """Standard ping-pong: scores -> exp -> PV per group."""
                pv = [aux_psp.tile([65, 512], F32, tag="aux", name=f"pv{h01}")
                      for h01 in (0, 1)]
                for g0 in range(0, 4 * qc, 2):
                    kcs = list(range(g0, min(g0 + 2, 4 * qc)))
                    pts = emit_scores_offdiag(p, qc, kcs)
                    emit_pv_offdiag(p, qc, kcs, pts, pv)
                for half, segs in enumerate(DIAG_SEGS):
                    pts = emit_scores_diag(p, qc, segs)
                    emit_pv_diag(p, qc, segs, pts, pv, last=(half == 1))
                emit_norm(p, qc, pv)

            # (p1, qc3) scores zipped against vproj; pt tiles held
            held1 = []
            vprog = 0
            for gi, (kind, arg, fn) in enumerate(score_group_closures(1, 3)):
                held1.append((kind, arg, fn()))
                while vprog < (gi + 1) * 2 and vprog < KC:
                    emit_vproj(vprog)
                    vprog += 1
            while vprog < KC:
                emit_vproj(vprog)
                vprog += 1

            emit_pv_all(0, 3, held0)
            emit_pv_all(1, 3, held1)
            emit_attn(2, 3)
            emit_outproj(3)
            for qc in (2, 1, 0):
                for p in range(NP):
                    emit_attn(p, qc)
                emit_outproj(qc)

    nc.compile()
    return nc


# make mybir importable inside _build_program's nested scopes
from concourse import mybir  # noqa: E402


def _get_compiled():
    global _COMPILED
    if _COMPILED is None:
        _COMPILED = _build_program()
    return _COMPILED


def _make_in_maps(inputs):
    import ml_dtypes

    BF = ml_dtypes.bfloat16
    x = np.asarray(inputs["x"], dtype=np.float32)
    wq = np.asarray(inputs["wq"], dtype=np.float32).astype(BF)
    wk = np.asarray(inputs["wk"], dtype=np.float32).astype(BF)
    wv = np.asarray(inputs["wv"], dtype=np.float32).astype(BF)
    wo = np.asarray(inputs["wo"], dtype=np.float32).astype(BF)

    ccat, scat = _rope_tables()
    r2t = _rot_matrix()
    m = np.arange(128)
    utri = (m[:, None] <= m[None, :]).astype(BF)
    eband = np.zeros((128, 128), dtype=np.float32)
    eband[np.arange(1, 128), np.arange(127)] = -1e9
    eband = eband.astype(BF)

    xTs = [np.ascontiguousarray(x[b].T.astype(BF)) for b in range(B)]
    in_maps = []
    for c in range(8):
        b, g = c // 2, c % 2
        sl = slice(g * HPC * D, (g + 1) * HPC * D)
        in_maps.append(dict(
            xT=xTs[b],
            wq=np.ascontiguousarray(wq[:, sl]),
            wk=np.ascontiguousarray(wk[:, sl]),
            wv=np.ascontiguousarray(wv[:, sl]),
            wo=np.ascontiguousarray(wo[sl, :]),
            ccat=ccat, scat=scat, r2t=r2t, utri=utri, eband=eband,
        ))
    return in_maps


def kernel(x, wq, wk, wv, wo, mask):
    """Full inputs in, full output out. Shards across 8 NeuronCores internally.

    The mask input is the standard causal mask produced by setup_inputs();
    causality is implemented directly on-device.
    """
    from concourse.bass_utils import run_bass_kernel_spmd

    in_maps = _make_in_maps(dict(x=x, wq=wq, wk=wk, wv=wv, wo=wo))

    nc = _get_compiled()
    res = run_bass_kernel_spmd(nc, in_maps, list(range(8)))
    out = np.empty((B, T, C), dtype=np.float32)
    for b in range(B):
        out[b] = res.results[2 * b]["y"] + res.results[2 * b + 1]["y"]
    return out


# revision 7
# speedup vs baseline: 1.1387x; 1.0749x over previous
"""Causal multi-head attention with RoPE for Trainium2, sharded over 8 NeuronCores.

Problem: B=4, T=2048, C=768, H=12, D=64, fp32 in/out.
    q,k,v = x @ wq/wk/wv  (per-head reshape), RoPE(q,k), causal softmax(q k^T/sqrt(D)) v,
    out = concat_heads @ wo.

Sharding: core c -> (batch b = c//2, head-group g = c%2 covering heads g*6..g*6+5).
Each core computes its 6 heads' attention and a partial output projection
y_c = out_heads(g) @ wo[rows g]; the host sums the two partials per batch.

On-core dataflow (bf16 matmul operands, fp32 PSUM accumulation; bf16 enables
Fast Weight Load so LDWEIGHTS mostly hides under the matmul streams):
  - host passes x^T (bf16) so every matmul contracts along partitions;
    x chunks are split across both DMA rings so phase 1 starts ~6us in.
  - q^T,k^T in [head_dim, T] layout; RoPE via a block-rotation matmul +
    cos/sin tensor ops on DVE; the rotation matmuls of block i-1 are
    emitted after block i's projection chains so the PSUM->SBUF copy
    latency never stalls the PE.
  - scores transposed: S^T[k, q] = k^T.T @ q^T with K=64 row-pairing
    (even head at partitions 0:64, odd at 64:128 -> concurrent row groups).
  - P = exp(S/8) on ScalarE -> bf16; causal masking of diagonal tiles via
    a bf16 utri@eband matmul add before exp.
  - PV with a ones-row appended to V: out_unnorm^T[d, q] and l[q] in one
    accumulated matmul chain per (head, q-chunk).
  - normalization: l row -> partition 0, gpsimd.partition_broadcast,
    reciprocal_approx_fast, TT multiply; the tiny qc=0 tail is emitted
    breadth-first across the three head-pairs to overlap its latency
    chains.
  - output projection accumulates 3 head-pair chunks into [128, 768] PSUM.
"""

import numpy as np
from contextlib import ExitStack

B, T, C, H, D = 4, 2048, 768, 12, 64
HPC = 6          # heads per core
NP = 3           # head-pair tiles per core
CC = C // 128    # 6 contraction chunks
TT = T // 128    # 16 t tiles
QC = T // 512    # 4 q chunks
KC = T // 128    # 16 k chunks

_COMPILED = None


def _rope_tables():
    import ml_dtypes
    inv_freq = 1.0 / (10000.0 ** (np.arange(0, D, 2, dtype=np.float64) / D))  # [32]
    t = np.arange(T, dtype=np.float64)
    freqs = np.outer(t, inv_freq)                      # [T, 32]
    cosT = np.cos(freqs).T.astype(np.float32)          # [32, T]
    sinT = np.sin(freqs).T.astype(np.float32)
    ccat = np.tile(cosT, (4, 1)).astype(ml_dtypes.bfloat16)   # [128, T]
    scat = np.tile(sinT, (4, 1)).astype(ml_dtypes.bfloat16)
    return np.ascontiguousarray(ccat), np.ascontiguousarray(scat)


def _rot_matrix():
    import ml_dtypes
    # rotate_half as a matmul: rot = R @ q (q in [D, T] layout), per 64-row block
    R = np.zeros((D, D), dtype=np.float32)
    R[0:32, 32:64] = -np.eye(32, dtype=np.float32)
    R[32:64, 0:32] = np.eye(32, dtype=np.float32)
    R2 = np.zeros((128, 128), dtype=np.float32)
    R2[0:64, 0:64] = R
    R2[64:128, 64:128] = R
    return np.ascontiguousarray(R2.T.astype(ml_dtypes.bfloat16))  # lhsT for out = R2 @ q


def _build_program():
    import concourse.tile as tile
    from concourse import bacc, mybir

    F32 = mybir.dt.float32
    BF16 = mybir.dt.bfloat16
    EXP = mybir.ActivationFunctionType.Exp

    nc = bacc.Bacc("TRN2", target_bir_lowering=False, debug=False, num_devices=8)

    xT_d = nc.dram_tensor("xT", [C, T], BF16, kind="ExternalInput").ap()
    wq_d = nc.dram_tensor("wq", [C, HPC * D], BF16, kind="ExternalInput").ap()
    wk_d = nc.dram_tensor("wk", [C, HPC * D], BF16, kind="ExternalInput").ap()
    wv_d = nc.dram_tensor("wv", [C, HPC * D], BF16, kind="ExternalInput").ap()
    wo_d = nc.dram_tensor("wo", [HPC * D, C], BF16, kind="ExternalInput").ap()
    ccat_d = nc.dram_tensor("ccat", [128, T], BF16, kind="ExternalInput").ap()
    scat_d = nc.dram_tensor("scat", [128, T], BF16, kind="ExternalInput").ap()
    r2t_d = nc.dram_tensor("r2t", [128, 128], BF16, kind="ExternalInput").ap()
    utri_d = nc.dram_tensor("utri", [128, 128], BF16, kind="ExternalInput").ap()
    eband_d = nc.dram_tensor("eband", [128, 128], BF16, kind="ExternalInput").ap()
    y_d = nc.dram_tensor("y", [T, C], F32, kind="ExternalOutput").ap()

    with tile.TileContext(nc) as tc, ExitStack() as ctx:
        big_pool = ctx.enter_context(tc.tile_pool(name="big", bufs=1))
        q_all = big_pool.tile([128, NP, T], BF16)
        k_all = big_pool.tile([128, NP, T], BF16)

        # ---- phase 0: input DMAs, critical-path first, split over rings ----
        cst_pool = ctx.enter_context(tc.tile_pool(name="cst", bufs=1))
        xt_pool = ctx.enter_context(tc.tile_pool(name="xt", bufs=1))
        xt_sb = xt_pool.tile([128, CC, T], BF16)

        r2t = cst_pool.tile([128, 128], BF16)
        nc.sync.dma_start(r2t[:], r2t_d)
        wv_sb = cst_pool.tile([128, CC, HPC * D], BF16)
        wo_sb = cst_pool.tile([128, NP, C], BF16)
        utri = cst_pool.tile([128, 128], BF16)
        eband = cst_pool.tile([128, 128], BF16)
        exp_warm = cst_pool.tile([1, 2], F32)

        with tc.tile_pool(name="w", bufs=1) as w_pool, \
             tc.tile_pool(name="const", bufs=1) as const_pool, \
             tc.tile_pool(name="p1ps", bufs=4, space="PSUM") as p1ps, \
             tc.tile_pool(name="p1tmp", bufs=2) as p1tmp:
            wq_sb = w_pool.tile([128, CC, HPC * D], BF16)
            nc.sync.dma_start(wq_sb[:], wq_d.rearrange("(cc p) d -> p cc d", p=128))
            wk_sb = w_pool.tile([128, CC, HPC * D], BF16)
            ccat = const_pool.tile([128, T], BF16)
            scat = const_pool.tile([128, T], BF16)
            nc.scalar.dma_start(wk_sb[:], wk_d.rearrange("(cc p) d -> p cc d", p=128))
            xT_r = xT_d.rearrange("(cc p) t -> p cc t", p=128)
            for cc in range(CC):
                eng = nc.sync if cc % 2 == 0 else nc.scalar
                eng.dma_start(xt_sb[:, cc, :], xT_r[:, cc, :])
            nc.sync.dma_start(ccat[:], ccat_d)
            nc.scalar.dma_start(scat[:], scat_d)
            nc.sync.dma_start(utri[:], utri_d)
            nc.scalar.dma_start(eband[:], eband_d)
            nc.sync.dma_start(wv_sb[:], wv_d.rearrange("(cc p) d -> p cc d", p=128))
            nc.scalar.dma_start(wo_sb[:], wo_d.rearrange("(hc p) c -> p hc c", p=128))

            # HAM warmup + Exp table preload while the input DMAs land
            warm_t = p1ps.tile([128, 1024], F32, tag="p1")
            warm = warm_t[:, 0:128]
            nc.scalar.activation(exp_warm[:], r2t[0:1, 0:2], EXP)
            for _ in range(16):
                nc.tensor.matmul(warm[:], r2t[:], r2t[:], start=True, stop=True)

            # ---- phase 1: q^T, k^T projections + RoPE, rot one block behind ----
            def finish_block(blk):
                dt, dst, qraw, sin_t = blk
                nc.vector.tensor_mul(dst[:, dt, :], qraw[:], ccat[:])
                for hh in range(2):
                    hsl = slice(hh * 1024, (hh + 1) * 1024)
                    ps_r = p1ps.tile([128, 1024], F32, tag="p1", name="ps_r")
                    for tq in range(2):
                        nc.tensor.matmul(
                            ps_r[:, tq * 512:(tq + 1) * 512],
                            r2t[:],
                            qraw[:, hh * 1024 + tq * 512:
                                  hh * 1024 + (tq + 1) * 512],
                            start=True, stop=True,
                        )
                    nc.vector.tensor_mul(sin_t[:, hsl], ps_r[:, :], scat[:, hsl])
                nc.vector.tensor_add(dst[:, dt, :], dst[:, dt, :], sin_t[:])

            prev = None
            for dt in range(NP):
                for w_sb, dst in ((wq_sb, q_all), (wk_sb, k_all)):
                    qraw = p1tmp.tile([128, T], BF16, tag="qraw")
                    sin_t = p1tmp.tile([128, T], BF16, tag="sin")
                    for hh in range(2):
                        hsl = slice(hh * 1024, (hh + 1) * 1024)
                        ps_q = p1ps.tile([128, 1024], F32, tag="p1", name="ps_q")
                        for cc in range(CC):
                            for tq in range(2):
                                nc.tensor.matmul(
                                    ps_q[:, tq * 512:(tq + 1) * 512],
                                    w_sb[:, cc, dt * 128:(dt + 1) * 128],
                                    xt_sb[:, cc,
                                          hh * 1024 + tq * 512:
                                          hh * 1024 + (tq + 1) * 512],
                                    start=(cc == 0), stop=(cc == CC - 1),
                                )
                        nc.scalar.copy(qraw[:, hsl], ps_q[:, :])
                    if prev is not None:
                        finish_block(prev)
                    prev = (dt, dst, qraw, sin_t)
            finish_block(prev)

        # ---- phase 2: fused v-projection + attention + output projection ----
        # PSUM (8 banks): s0,s1 [128,1024] (4) + aux rotation (4): ps_v/pv/y
        with tc.tile_pool(name="big2", bufs=1) as big2_pool, \
             tc.tile_pool(name="s_ps", bufs=1, space="PSUM") as s_psp, \
             tc.tile_pool(name="aux_ps", bufs=4, space="PSUM") as aux_psp, \
             tc.tile_pool(name="p_sb", bufs=6) as p_sbp, \
             tc.tile_pool(name="l_sb", bufs=3) as l_sbp, \
             tc.tile_pool(name="r_sb", bufs=3) as r_sbp, \
             tc.tile_pool(name="y_sb", bufs=2) as y_sbp:
            v_aug = big2_pool.tile([128, KC, HPC, D + 1], BF16)
            out_norm = big2_pool.tile([128, NP, T], BF16)
            nc.gpsimd.memset(v_aug[:, :, :, D:D + 1], 1.0)

            def emit_vproj(tt):
                ps_v = aux_psp.tile([128, HPC * D], F32, tag="aux", name="ps_v")
                for cc in range(CC):
                    nc.tensor.matmul(
                        ps_v[:, 0:HPC * D],
                        xt_sb[:, cc, tt * 128:(tt + 1) * 128],
                        wv_sb[:, cc, :],
                        start=(cc == 0), stop=(cc == CC - 1),
                    )
                nc.vector.tensor_copy(
                    v_aug[:, tt, :, 0:D],
                    ps_v[:, 0:HPC * D].rearrange("p (h d) -> p h d", d=D),
                )

            def emit_scores_offdiag(p, qc, kcs):
                s_t = [s_psp.tile([128, 1024], F32, tag=f"s{h01}",
                                  name=f"s_t{h01}") for h01 in (0, 1)]
                for j, kc in enumerate(kcs):
                    for h01 in (0, 1):
                        r0, r1 = h01 * 64, h01 * 64 + 64
                        nc.tensor.matmul(
                            s_t[h01][:, j * 512:(j + 1) * 512],
                            k_all[r0:r1, p, kc * 128:(kc + 1) * 128],
                            q_all[r0:r1, p, qc * 512:(qc + 1) * 512],
                            start=True, stop=True,
                        )
                pts = []
                for h01 in (0, 1):
                    pt = p_sbp.tile([128, 1024], BF16, tag=f"pt{h01}")
                    w = len(kcs) * 512
                    nc.scalar.activation(pt[:, 0:w], s_t[h01][:, 0:w], EXP,
                                         scale=0.125)
                    pts.append(pt)
                return pts

            def emit_pv_offdiag(p, qc, kcs, pts, pv):
                for j, kc in enumerate(kcs):
                    for h01 in (0, 1):
                        nc.tensor.matmul(
                            pv[h01][:],
                            v_aug[:, kc, p * 2 + h01, :],
                            pts[h01][:, j * 512:(j + 1) * 512],
                            start=(kc == 0), stop=False,
                        )

            # diagonal tiles: half 0 = j0(512)+j1(384), half 1 = j2(256)+j3(128)
            DIAG_SEGS = (((0, 0, 512), (1, 512, 384)),
                         ((2, 0, 256), (3, 256, 128)))

            def emit_scores_diag(p, qc, segs):
                s_d = [s_psp.tile([128, 1024], F32, tag=f"s{h01}",
                                  name=f"s_d{h01}") for h01 in (0, 1)]
                for j, off, wj in segs:
                    kc = 4 * qc + j
                    for h01 in (0, 1):
                        r0, r1 = h01 * 64, h01 * 64 + 64
                        nc.tensor.matmul(
                            s_d[h01][:, off:off + wj],
                            k_all[r0:r1, p, kc * 128:(kc + 1) * 128],
                            q_all[r0:r1, p, qc * 512 + 128 * j:qc * 512 + 512],
                            start=True, stop=False,
                        )
                    for h01 in (0, 1):
                        nc.tensor.matmul(
                            s_d[h01][:, off:off + 128],
                            utri[:], eband[:],
                            start=False, stop=True,
                        )
                pts = []
                for h01 in (0, 1):
                    pt_d = p_sbp.tile([128, 1024], BF16, tag=f"pt{h01}",
                                      name="pt_d")
                    wtot = sum(sg[2] for sg in segs)
                    nc.scalar.activation(pt_d[:, 0:wtot], s_d[h01][:, 0:wtot],
                                         EXP, scale=0.125)
                    pts.append(pt_d)
                return pts

            def emit_pv_diag(p, qc, segs, pts, pv, last):
                for j, off, wj in segs:
                    kc = 4 * qc + j
                    for h01 in (0, 1):
                        nc.tensor.matmul(
                            pv[h01][:, 128 * j:512],
                            v_aug[:, kc, p * 2 + h01, :],
                            pts[h01][:, off:off + wj],
                            start=(kc == 0), stop=(last and j == 3),
                        )

            def emit_norm(p, qc, pv):
                for h01 in (0, 1):
                    lrow = l_sbp.tile([1, 512], F32, tag=f"l{h01}")
                    nc.vector.tensor_copy(lrow[0:1, :], pv[h01][64:65, :])
                    rbc = r_sbp.tile([64, 512], F32, tag=f"r{h01}")
                    nc.gpsimd.partition_broadcast(rbc[:], lrow[0:1, :],
                                                  channels=64)
                    nc.vector.reciprocal_approx_fast(rbc[:], rbc[:])
                    nc.vector.tensor_mul(
                        out_norm[h01 * 64:h01 * 64 + 64, p,
                                 qc * 512:(qc + 1) * 512],
                        pv[h01][0:64, :],
                        rbc[:],
                    )

            def emit_attn(p, qc, pv):
                for g0 in range(0, 4 * qc, 2):
                    kcs = list(range(g0, min(g0 + 2, 4 * qc)))
                    pts = emit_scores_offdiag(p, qc, kcs)
                    emit_pv_offdiag(p, qc, kcs, pts, pv)
                for half, segs in enumerate(DIAG_SEGS):
                    pts = emit_scores_diag(p, qc, segs)
                    emit_pv_diag(p, qc, segs, pts, pv, last=(half == 1))
                emit_norm(p, qc, pv)

            def emit_outproj(qc):
                for tt in range(4 * qc, 4 * qc + 4):
                    y_a = aux_psp.tile([128, 512], F32, tag="aux", name="y_a")
                    y_b = aux_psp.tile([128, 256], F32, tag="aux", name="y_b")
                    for hc in range(NP):
                        lhsT = out_norm[:, hc, tt * 128:(tt + 1) * 128]
                        nc.tensor.matmul(y_a[:, 0:512], lhsT,
                                         wo_sb[:, hc, 0:512],
                                         start=(hc == 0), stop=(hc == NP - 1))
                        nc.tensor.matmul(y_b[:, 0:256], lhsT,
                                         wo_sb[:, hc, 512:768],
                                         start=(hc == 0), stop=(hc == NP - 1))
                    yt = y_sbp.tile([128, C], F32, tag="yt")
                    nc.vector.tensor_copy(yt[:, 0:512], y_a[:, 0:512])
                    nc.vector.tensor_copy(yt[:, 512:768], y_b[:, 0:256])
                    nc.sync.dma_start(y_d[tt * 128:(tt + 1) * 128, :], yt[:])

            for qi, qc in enumerate((3, 2, 1)):
                for tt in (range(KC) if qi == 0 else ()):
                    emit_vproj(tt)
                for p in range(NP):
                    pv = [aux_psp.tile([65, 512], F32, tag="aux",
                                       name=f"pv{h01}") for h01 in (0, 1)]
                    emit_attn(p, qc, pv)
                emit_outproj(qc)

            # qc=0 tail: breadth-first across head-pairs to overlap the
            # short latency chains (scores/exp/PV/norm are tiny here)
            qc = 0
            pvs, helds = [], []
            for p in range(NP):
                helds.append([(segs, emit_scores_diag(p, qc, segs))
                              for segs in DIAG_SEGS])
            for p in range(NP):
                pv = [aux_psp.tile([65, 512], F32, tag="aux",
                                   name=f"pv{h01}") for h01 in (0, 1)]
                for half, (segs, pts) in enumerate(helds[p]):
                    emit_pv_diag(p, qc, segs, pts, pv, last=(half == 1))
                emit_norm(p, qc, pv)
            emit_outproj(0)

    nc.compile()
    return nc


# make mybir importable inside _build_program's nested scopes
from concourse import mybir  # noqa: E402


def _get_compiled():
    global _COMPILED
    if _COMPILED is None:
        _COMPILED = _build_program()
    return _COMPILED


def _make_in_maps(inputs):
    import ml_dtypes

    BF = ml_dtypes.bfloat16
    x = np.asarray(inputs["x"], dtype=np.float32)
    wq = np.asarray(inputs["wq"], dtype=np.float32).astype(BF)
    wk = np.asarray(inputs["wk"], dtype=np.float32).astype(BF)
    wv = np.asarray(inputs["wv"], dtype=np.float32).astype(BF)
    wo = np.asarray(inputs["wo"], dtype=np.float32).astype(BF)

    ccat, scat = _rope_tables()
    r2t = _rot_matrix()
    m = np.arange(128)
    utri = (m[:, None] <= m[None, :]).astype(BF)
    eband = np.zeros((128, 128), dtype=np.float32)
    eband[np.arange(1, 128), np.arange(127)] = -1e9
    eband = eband.astype(BF)

    xTs = [np.ascontiguousarray(x[b].T.astype(BF)) for b in range(B)]
    in_maps = []
    for c in range(8):
        b, g = c // 2, c % 2
        sl = slice(g * HPC * D, (g + 1) * HPC * D)
        in_maps.append(dict(
            xT=xTs[b],
            wq=np.ascontiguousarray(wq[:, sl]),
            wk=np.ascontiguousarray(wk[:, sl]),
            wv=np.ascontiguousarray(wv[:, sl]),
            wo=np.ascontiguousarray(wo[sl, :]),
            ccat=ccat, scat=scat, r2t=r2t, utri=utri, eband=eband,
        ))
    return in_maps


def kernel(x, wq, wk, wv, wo, mask):
    """Full inputs in, full output out. Shards across 8 NeuronCores internally.

    The mask input is the standard causal mask produced by setup_inputs();
    causality is implemented directly on-device.
    """
    from concourse.bass_utils import run_bass_kernel_spmd

    in_maps = _make_in_maps(dict(x=x, wq=wq, wk=wk, wv=wv, wo=wo))

    nc = _get_compiled()
    res = run_bass_kernel_spmd(nc, in_maps, list(range(8)))
    out = np.empty((B, T, C), dtype=np.float32)
    for b in range(B):
        out[b] = res.results[2 * b]["y"] + res.results[2 * b + 1]["y"]
    return out


# revision 13
# speedup vs baseline: 1.3185x; 1.1579x over previous
"""Causal multi-head attention with RoPE for Trainium2, sharded over 8 NeuronCores.

Problem: B=4, T=2048, C=768, H=12, D=64, fp32 in/out.
    q,k,v = x @ wq/wk/wv  (per-head reshape), RoPE(q,k), causal softmax(q k^T/sqrt(D)) v,
    out = concat_heads @ wo.

Sharding: core c -> (batch b = c//2, head-group g = c%2 covering heads g*6..g*6+5).
Each core computes its 6 heads' attention and a partial output projection
y_c = out_heads(g) @ wo[rows g]; the host sums the two partials per batch.

On-core dataflow (bf16 matmul operands, fp32 PSUM accumulation; bf16 enables
Fast Weight Load so LDWEIGHTS mostly hides under the matmul streams):
  - host passes x^T (bf16) so every matmul contracts along partitions;
    x chunks are split across both DMA rings so phase 1 starts ~6us in.
  - q^T,k^T in [head_dim, T] layout; RoPE via a block-rotation matmul +
    cos/sin tensor ops on DVE; the rotation matmuls of block i-1 are
    emitted after block i's projection chains so the PSUM->SBUF copy
    latency never stalls the PE.
  - scores transposed: S^T[k, q] = k^T.T @ q^T with K=64 row-pairing
    (even head at partitions 0:64, odd at 64:128 -> concurrent row groups).
  - P = exp(S/8) on ScalarE -> bf16; causal masking of diagonal tiles via
    a bf16 utri@eband matmul add before exp.
  - PV with a ones-row appended to V: out_unnorm^T[d, q] and l[q] in one
    accumulated matmul chain per (head, q-chunk).
  - normalization: l row -> partition 0, gpsimd.partition_broadcast,
    reciprocal_approx_fast, TT multiply; the tiny qc=0 tail is emitted
    breadth-first across the three head-pairs to overlap its latency
    chains.
  - output projection accumulates 3 head-pair chunks into [128, 768] PSUM.
"""

import numpy as np
from contextlib import ExitStack

B, T, C, H, D = 4, 2048, 768, 12, 64
HPC = 6          # heads per core
NP = 3           # head-pair tiles per core
CC = C // 128    # 6 contraction chunks
TT = T // 128    # 16 t tiles
QC = T // 512    # 4 q chunks
KC = T // 128    # 16 k chunks

_COMPILED = None


def _rope_tables():
    import ml_dtypes
    inv_freq = 1.0 / (10000.0 ** (np.arange(0, D, 2, dtype=np.float64) / D))  # [32]
    t = np.arange(T, dtype=np.float64)
    freqs = np.outer(t, inv_freq)                      # [T, 32]
    cosT = np.cos(freqs).T.astype(np.float32)          # [32, T]
    sinT = np.sin(freqs).T.astype(np.float32)
    ccat = np.tile(cosT, (4, 1)).astype(ml_dtypes.bfloat16)   # [128, T]
    scat = np.tile(sinT, (4, 1)).astype(ml_dtypes.bfloat16)
    return np.ascontiguousarray(ccat), np.ascontiguousarray(scat)


def _rot_matrix():
    import ml_dtypes
    # rotate_half as a matmul: rot = R @ q (q in [D, T] layout), per 64-row block
    R = np.zeros((D, D), dtype=np.float32)
    R[0:32, 32:64] = -np.eye(32, dtype=np.float32)
    R[32:64, 0:32] = np.eye(32, dtype=np.float32)
    R2 = np.zeros((128, 128), dtype=np.float32)
    R2[0:64, 0:64] = R
    R2[64:128, 64:128] = R
    return np.ascontiguousarray(R2.T.astype(ml_dtypes.bfloat16))  # lhsT for out = R2 @ q


def _build_program():
    import concourse.tile as tile
    from concourse import bacc, mybir

    F32 = mybir.dt.float32
    BF16 = mybir.dt.bfloat16
    EXP = mybir.ActivationFunctionType.Exp

    nc = bacc.Bacc("TRN2", target_bir_lowering=False, debug=False, num_devices=8)

    xT_d = nc.dram_tensor("xT", [C, T], BF16, kind="ExternalInput").ap()
    wq_d = nc.dram_tensor("wq", [C, HPC * D], BF16, kind="ExternalInput").ap()
    wk_d = nc.dram_tensor("wk", [C, HPC * D], BF16, kind="ExternalInput").ap()
    wv_d = nc.dram_tensor("wv", [C, HPC * D], BF16, kind="ExternalInput").ap()
    wo_d = nc.dram_tensor("wo", [HPC * D, C], BF16, kind="ExternalInput").ap()
    ccat_d = nc.dram_tensor("ccat", [128, T], BF16, kind="ExternalInput").ap()
    scat_d = nc.dram_tensor("scat", [128, T], BF16, kind="ExternalInput").ap()
    r2t_d = nc.dram_tensor("r2t", [128, 128], BF16, kind="ExternalInput").ap()
    utri_d = nc.dram_tensor("utri", [128, 128], BF16, kind="ExternalInput").ap()
    eband_d = nc.dram_tensor("eband", [128, 128], BF16, kind="ExternalInput").ap()
    y_d = nc.dram_tensor("y", [T, C], F32, kind="ExternalOutput").ap()

    with tile.TileContext(nc) as tc, ExitStack() as ctx:
        big_pool = ctx.enter_context(tc.tile_pool(name="big", bufs=1))
        q_all = big_pool.tile([128, NP, T], BF16)
        k_all = big_pool.tile([128, NP, T], BF16)

        # ---- phase 0: input DMAs, critical-path first, split over rings ----
        cst_pool = ctx.enter_context(tc.tile_pool(name="cst", bufs=1))
        xt_pool = ctx.enter_context(tc.tile_pool(name="xt", bufs=1))
        xt_sb = xt_pool.tile([128, CC, T], BF16)

        r2t = cst_pool.tile([128, 128], BF16)
        nc.sync.dma_start(r2t[:], r2t_d)
        wv_sb = cst_pool.tile([128, CC, HPC * D], BF16)
        wo_sb = cst_pool.tile([128, NP, C], BF16)
        utri = cst_pool.tile([128, 128], BF16)
        eband = cst_pool.tile([128, 128], BF16)
        exp_warm = cst_pool.tile([1, 2], F32)

        with tc.tile_pool(name="w", bufs=1) as w_pool, \
             tc.tile_pool(name="const", bufs=1) as const_pool, \
             tc.tile_pool(name="p1ps", bufs=4, space="PSUM") as p1ps, \
             tc.tile_pool(name="p1tmp", bufs=2) as p1tmp:
            wq_sb = w_pool.tile([128, CC, HPC * D], BF16)
            nc.sync.dma_start(wq_sb[:], wq_d.rearrange("(cc p) d -> p cc d", p=128))
            wk_sb = w_pool.tile([128, CC, HPC * D], BF16)
            ccat = const_pool.tile([128, T], BF16)
            scat = const_pool.tile([128, T], BF16)
            nc.scalar.dma_start(wk_sb[:], wk_d.rearrange("(cc p) d -> p cc d", p=128))
            xT_r = xT_d.rearrange("(cc p) t -> p cc t", p=128)
            x_engs = (nc.gpsimd, nc.sync, nc.scalar, nc.gpsimd, nc.sync,
                      nc.scalar)
            for cc in range(CC):
                x_engs[cc].dma_start(xt_sb[:, cc, :], xT_r[:, cc, :])
            nc.sync.dma_start(ccat[:], ccat_d)
            nc.scalar.dma_start(scat[:], scat_d)
            nc.sync.dma_start(utri[:], utri_d)
            nc.scalar.dma_start(eband[:], eband_d)
            nc.gpsimd.dma_start(wv_sb[:], wv_d.rearrange("(cc p) d -> p cc d", p=128))
            nc.gpsimd.dma_start(wo_sb[:], wo_d.rearrange("(hc p) c -> p hc c", p=128))

            # HAM warmup + Exp table preload while the input DMAs land
            warm_t = p1ps.tile([128, 1024], F32, tag="p1")
            warm = warm_t[:, 0:128]
            nc.scalar.activation(exp_warm[:], r2t[0:1, 0:2], EXP)
            for _ in range(16):
                nc.tensor.matmul(warm[:], r2t[:], r2t[:], start=True, stop=True)
            # rotation-phase shims: ensure the LAST p1 allocs (freed late, by
            # the trailing DVE sin-muls) land on the banks that phase 2
            # touches last (aux), and the early-freed psq banks host s0/s1
            for _ in range(3):
                p1ps.tile([128, 1024], F32, tag="p1", name="shim")

            # ---- phase 1: q^T, k^T projections + RoPE, rot one block behind ----
            def finish_block(blk):
                dt, dst, qraw, sin_t = blk
                nc.vector.tensor_mul(dst[:, dt, :], qraw[:], ccat[:])
                for hh in range(2):
                    hsl = slice(hh * 1024, (hh + 1) * 1024)
                    ps_r = p1ps.tile([128, 1024], F32, tag="p1", name="ps_r")
                    for tq in range(2):
                        nc.tensor.matmul(
                            ps_r[:, tq * 512:(tq + 1) * 512],
                            r2t[:],
                            qraw[:, hh * 1024 + tq * 512:
                                  hh * 1024 + (tq + 1) * 512],
                            start=True, stop=True,
                        )
                    nc.vector.tensor_mul(sin_t[:, hsl], ps_r[:, :], scat[:, hsl])
                nc.vector.tensor_add(dst[:, dt, :], dst[:, dt, :], sin_t[:])

            prev = None
            for dt in range(NP):
                for w_sb, dst in ((wq_sb, q_all), (wk_sb, k_all)):
                    qraw = p1tmp.tile([128, T], BF16, tag="qraw")
                    sin_t = p1tmp.tile([128, T], BF16, tag="sin")
                    for hh in range(2):
                        hsl = slice(hh * 1024, (hh + 1) * 1024)
                        ps_q = p1ps.tile([128, 1024], F32, tag="p1", name="ps_q")
                        for cc in range(CC):
                            for tq in range(2):
                                nc.tensor.matmul(
                                    ps_q[:, tq * 512:(tq + 1) * 512],
                                    w_sb[:, cc, dt * 128:(dt + 1) * 128],
                                    xt_sb[:, cc,
                                          hh * 1024 + tq * 512:
                                          hh * 1024 + (tq + 1) * 512],
                                    start=(cc == 0), stop=(cc == CC - 1),
                                )
                        nc.scalar.copy(qraw[:, hsl], ps_q[:, :])
                    if prev is not None:
                        finish_block(prev)
                    prev = (dt, dst, qraw, sin_t)
            finish_block(prev)

        # ---- phase 2: fused v-projection + attention + output projection ----
        # PSUM (8 banks): s0,s1 [128,1024] (4) + aux rotation (4): ps_v/pv/y
        with tc.tile_pool(name="big2", bufs=1) as big2_pool, \
             tc.tile_pool(name="s_ps", bufs=1, space="PSUM") as s_psp, \
             tc.tile_pool(name="aux_ps", bufs=4, space="PSUM") as aux_psp, \
             tc.tile_pool(name="p_sb", bufs=12) as p_sbp, \
             tc.tile_pool(name="l_sb", bufs=3) as l_sbp, \
             tc.tile_pool(name="r_sb", bufs=3) as r_sbp, \
             tc.tile_pool(name="y_sb", bufs=2) as y_sbp:
            v_aug = big2_pool.tile([128, KC, HPC, D + 1], BF16)
            out_norm = big2_pool.tile([128, NP, T], BF16)
            nc.gpsimd.memset(v_aug[:, :, :, D:D + 1], 1.0)

            def emit_vproj(tt):
                ps_v = aux_psp.tile([128, HPC * D], F32, tag="aux", name="ps_v")
                for cc in range(CC):
                    nc.tensor.matmul(
                        ps_v[:, 0:HPC * D],
                        xt_sb[:, cc, tt * 128:(tt + 1) * 128],
                        wv_sb[:, cc, :],
                        start=(cc == 0), stop=(cc == CC - 1),
                    )
                nc.vector.tensor_copy(
                    v_aug[:, tt, :, 0:D],
                    ps_v[:, 0:HPC * D].rearrange("p (h d) -> p h d", d=D),
                )

            def emit_scores_offdiag(p, qc, kcs):
                s_t = [s_psp.tile([128, 1024], F32, tag=f"s{h01}",
                                  name=f"s_t{h01}") for h01 in (0, 1)]
                for j, kc in enumerate(kcs):
                    for h01 in (0, 1):
                        r0, r1 = h01 * 64, h01 * 64 + 64
                        nc.tensor.matmul(
                            s_t[h01][:, j * 512:(j + 1) * 512],
                            k_all[r0:r1, p, kc * 128:(kc + 1) * 128],
                            q_all[r0:r1, p, qc * 512:(qc + 1) * 512],
                            start=True, stop=True,
                        )
                pts = []
                for h01 in (0, 1):
                    pt = p_sbp.tile([128, 1024], BF16, tag=f"pt{h01}")
                    w = len(kcs) * 512
                    nc.scalar.activation(pt[:, 0:w], s_t[h01][:, 0:w], EXP,
                                         scale=0.125)
                    pts.append(pt)
                return pts

            def emit_pv_offdiag(p, qc, kcs, pts, pv):
                for j, kc in enumerate(kcs):
                    for h01 in (0, 1):
                        nc.tensor.matmul(
                            pv[h01][:],
                            v_aug[:, kc, p * 2 + h01, :],
                            pts[h01][:, j * 512:(j + 1) * 512],
                            start=(kc == 0), stop=False,
                        )

            # diagonal tiles: half 0 = j0(512)+j1(384), half 1 = j2(256)+j3(128)
            DIAG_SEGS = (((0, 0, 512), (1, 512, 384)),
                         ((2, 0, 256), (3, 256, 128)))

            def emit_scores_diag(p, qc, segs):
                s_d = [s_psp.tile([128, 1024], F32, tag=f"s{h01}",
                                  name=f"s_d{h01}") for h01 in (0, 1)]
                for j, off, wj in segs:
                    kc = 4 * qc + j
                    for h01 in (0, 1):
                        r0, r1 = h01 * 64, h01 * 64 + 64
                        nc.tensor.matmul(
                            s_d[h01][:, off:off + wj],
                            k_all[r0:r1, p, kc * 128:(kc + 1) * 128],
                            q_all[r0:r1, p, qc * 512 + 128 * j:qc * 512 + 512],
                            start=True, stop=False,
                        )
                    for h01 in (0, 1):
                        nc.tensor.matmul(
                            s_d[h01][:, off:off + 128],
                            utri[:], eband[:],
                            start=False, stop=True,
                        )
                pts = []
                for h01 in (0, 1):
                    pt_d = p_sbp.tile([128, 1024], BF16, tag=f"pt{h01}",
                                      name="pt_d")
                    wtot = sum(sg[2] for sg in segs)
                    nc.scalar.activation(pt_d[:, 0:wtot], s_d[h01][:, 0:wtot],
                                         EXP, scale=0.125)
                    pts.append(pt_d)
                return pts

            def emit_pv_diag(p, qc, segs, pts, pv, last):
                for j, off, wj in segs:
                    kc = 4 * qc + j
                    for h01 in (0, 1):
                        nc.tensor.matmul(
                            pv[h01][:, 128 * j:512],
                            v_aug[:, kc, p * 2 + h01, :],
                            pts[h01][:, off:off + wj],
                            start=(kc == 0), stop=(last and j == 3),
                        )

            def emit_norm(p, qc, pv):
                for h01 in (0, 1):
                    lrow = l_sbp.tile([1, 512], F32, tag=f"l{h01}")
                    nc.vector.tensor_copy(lrow[0:1, :], pv[h01][64:65, :])
                    rbc = r_sbp.tile([64, 512], F32, tag=f"r{h01}")
                    nc.gpsimd.partition_broadcast(rbc[:], lrow[0:1, :],
                                                  channels=64)
                    nc.vector.reciprocal_approx_fast(rbc[:], rbc[:])
                    nc.vector.tensor_mul(
                        out_norm[h01 * 64:h01 * 64 + 64, p,
                                 qc * 512:(qc + 1) * 512],
                        pv[h01][0:64, :],
                        rbc[:],
                    )

            def attn_units(p, qc):
                """(score_emitter, pv_emitter) pairs in kc order."""
                units = []
                for g0 in range(0, 4 * qc, 2):
                    kcs = list(range(g0, min(g0 + 2, 4 * qc)))
                    units.append((
                        (lambda kk: lambda: emit_scores_offdiag(p, qc, kk))(kcs),
                        (lambda kk: lambda pts, pv: emit_pv_offdiag(
                            p, qc, kk, pts, pv))(kcs),
                    ))
                for half, segs in enumerate(DIAG_SEGS):
                    units.append((
                        (lambda ss: lambda: emit_scores_diag(p, qc, ss))(segs),
                        (lambda ss, la: lambda pts, pv: emit_pv_diag(
                            p, qc, ss, pts, pv, last=la))(segs, half == 1),
                    ))
                return units

            def emit_attn(p, qc, pv, fillers=None):
                """Ping-pong with 1-group score lookahead; optional PE filler
                work (e.g. vproj closures) interleaved between groups."""
                fillers = list(fillers or [])
                fi = 0
                pending = None  # (pv_fn, pts)
                for si, (sc_fn, pv_fn) in enumerate(attn_units(p, qc)):
                    pts = sc_fn()
                    if pending is not None:
                        pending[0](pending[1], pv)
                    pending = (pv_fn, pts)
                    while fi < len(fillers) and fi < (si + 1) * 2:
                        fillers[fi]()
                        fi += 1
                while fi < len(fillers):
                    fillers[fi]()
                    fi += 1
                pending[0](pending[1], pv)
                emit_norm(p, qc, pv)

            def emit_outproj(qc):
                for tt in range(4 * qc, 4 * qc + 4):
                    y_a = aux_psp.tile([128, 512], F32, tag="aux", name="y_a")
                    y_b = aux_psp.tile([128, 256], F32, tag="aux", name="y_b")
                    for hc in range(NP):
                        lhsT = out_norm[:, hc, tt * 128:(tt + 1) * 128]
                        nc.tensor.matmul(y_a[:, 0:512], lhsT,
                                         wo_sb[:, hc, 0:512],
                                         start=(hc == 0), stop=(hc == NP - 1))
                        nc.tensor.matmul(y_b[:, 0:256], lhsT,
                                         wo_sb[:, hc, 512:768],
                                         start=(hc == 0), stop=(hc == NP - 1))
                    yt = y_sbp.tile([128, C], F32, tag="yt")
                    nc.vector.tensor_copy(yt[:, 0:512], y_a[:, 0:512])
                    nc.vector.tensor_copy(yt[:, 512:768], y_b[:, 0:256])
                    nc.sync.dma_start(y_d[tt * 128:(tt + 1) * 128, :], yt[:])

            # qc=3: the p=0 scores/exps zip against the v-projection so the
            # exp stream starts right as phase 1 drains
            vproj_fillers = [(lambda t: lambda: emit_vproj(t))(tt)
                             for tt in range(KC)]
            for p in range(NP):
                pv = [aux_psp.tile([65, 512], F32, tag="aux",
                                   name=f"pv{h01}") for h01 in (0, 1)]
                emit_attn(p, 3, pv, fillers=(vproj_fillers if p == 0 else None))
            emit_outproj(3)
            for p in range(NP):
                pv = [aux_psp.tile([65, 512], F32, tag="aux",
                                   name=f"pv{h01}") for h01 in (0, 1)]
                emit_attn(p, 2, pv)
            emit_outproj(2)

            # qc=1 and qc=0 tails: breadth-first across head-pairs to
            # overlap the short scores/exp/PV/norm latency chains
            for qc in (1, 0):
                helds = []
                for p in range(NP):
                    helds.append([(pv_fn, sc_fn())
                                  for sc_fn, pv_fn in attn_units(p, qc)])
                for p in range(NP):
                    pv = [aux_psp.tile([65, 512], F32, tag="aux",
                                       name=f"pv{h01}") for h01 in (0, 1)]
                    for pv_fn, pts in helds[p]:
                        pv_fn(pts, pv)
                    emit_norm(p, qc, pv)
                emit_outproj(qc)

    nc.compile()
    return nc


# make mybir importable inside _build_program's nested scopes
from concourse import mybir  # noqa: E402


def _get_compiled():
    global _COMPILED
    if _COMPILED is None:
        _COMPILED = _build_program()
    return _COMPILED


def _make_in_maps(inputs):
    import ml_dtypes

    BF = ml_dtypes.bfloat16
    x = np.asarray(inputs["x"], dtype=np.float32)
    wq = np.asarray(inputs["wq"], dtype=np.float32).astype(BF)
    wk = np.asarray(inputs["wk"], dtype=np.float32).astype(BF)
    wv = np.asarray(inputs["wv"], dtype=np.float32).astype(BF)
    wo = np.asarray(inputs["wo"], dtype=np.float32).astype(BF)

    ccat, scat = _rope_tables()
    r2t = _rot_matrix()
    m = np.arange(128)
    utri = (m[:, None] <= m[None, :]).astype(BF)
    eband = np.zeros((128, 128), dtype=np.float32)
    eband[np.arange(1, 128), np.arange(127)] = -1e9
    eband = eband.astype(BF)

    xTs = [np.ascontiguousarray(x[b].T.astype(BF)) for b in range(B)]
    in_maps = []
    for c in range(8):
        b, g = c // 2, c % 2
        sl = slice(g * HPC * D, (g + 1) * HPC * D)
        in_maps.append(dict(
            xT=xTs[b],
            wq=np.ascontiguousarray(wq[:, sl]),
            wk=np.ascontiguousarray(wk[:, sl]),
            wv=np.ascontiguousarray(wv[:, sl]),
            wo=np.ascontiguousarray(wo[sl, :]),
            ccat=ccat, scat=scat, r2t=r2t, utri=utri, eband=eband,
        ))
    return in_maps


def kernel(x, wq, wk, wv, wo, mask):
    """Full inputs in, full output out. Shards across 8 NeuronCores internally.

    The mask input is the standard causal mask produced by setup_inputs();
    causality is implemented directly on-device.
    """
    from concourse.bass_utils import run_bass_kernel_spmd

    in_maps = _make_in_maps(dict(x=x, wq=wq, wk=wk, wv=wv, wo=wo))

    nc = _get_compiled()
    res = run_bass_kernel_spmd(nc, in_maps, list(range(8)))
    out = np.empty((B, T, C), dtype=np.float32)
    for b in range(B):
        out[b] = res.results[2 * b]["y"] + res.results[2 * b + 1]["y"]
    return out


# revision 20
# speedup vs baseline: 1.3220x; 1.0027x over previous
"""Causal multi-head attention with RoPE for Trainium2, sharded over 8 NeuronCores.

Problem: B=4, T=2048, C=768, H=12, D=64, fp32 in/out.
    q,k,v = x @ wq/wk/wv  (per-head reshape), RoPE(q,k), causal softmax(q k^T/sqrt(D)) v,
    out = concat_heads @ wo.

Sharding: core c -> (batch b = c//2, head-group g = c%2 covering heads g*6..g*6+5).
Each core computes its 6 heads' attention and a partial output projection
y_c = out_heads(g) @ wo[rows g]; the host sums the two partials per batch.

On-core dataflow (bf16 matmul operands, fp32 PSUM accumulation; bf16 enables
Fast Weight Load and 1024-wide moving operands, so the q/k projection and
rotation chains use half the matmul instructions):
  - host passes x^T (bf16); input DMAs split across all three DGE queues,
    critical tensors first; a HAM-warmup matmul stream covers the load.
  - q^T,k^T in [head_dim, T] layout; RoPE via a block-rotation matmul +
    cos/sin tensor ops on DVE, pipelined one block behind the projection
    so the PSUM->SBUF copy latency never stalls the PE.
  - scores transposed: S^T[k, q] = k^T.T @ q^T with K=64 row-pairing
    (even head at partitions 0:64, odd at 64:128 -> concurrent row groups).
    The first two (p=0, qc=3) score groups are emitted inside the phase-1
    tail and the rest zip against the v-projection, so the ScalarE exp
    stream (the second serial resource, ~13.4M exps/core) starts the
    moment phase 1 drains.
  - P = exp(S/8) on ScalarE -> bf16; causal masking of diagonal tiles via
    a bf16 utri@eband matmul add before exp.
  - PV with a ones-row appended to V: out_unnorm^T[d, q] and l[q] in one
    accumulated matmul chain per (head, q-chunk); 1-group score lookahead
    keeps the PE ahead of the exp stream.
  - normalization: l row -> partition 0, gpsimd.partition_broadcast,
    reciprocal_approx_fast, TT multiply.  The small qc=1/qc=0 tails run
    breadth-first across head-pairs with their PSUM->SBUF copies moved to
    the by-then idle ScalarE.
  - output projection accumulates 3 head-pair chunks into [128, 768] PSUM.
"""

import numpy as np
from contextlib import ExitStack

B, T, C, H, D = 4, 2048, 768, 12, 64
HPC = 6          # heads per core
NP = 3           # head-pair tiles per core
CC = C // 128    # 6 contraction chunks
TT = T // 128    # 16 t tiles
QC = T // 512    # 4 q chunks
KC = T // 128    # 16 k chunks

_COMPILED = None


def _rope_tables():
    import ml_dtypes
    inv_freq = 1.0 / (10000.0 ** (np.arange(0, D, 2, dtype=np.float64) / D))  # [32]
    t = np.arange(T, dtype=np.float64)
    freqs = np.outer(t, inv_freq)                      # [T, 32]
    cosT = np.cos(freqs).T.astype(np.float32)          # [32, T]
    sinT = np.sin(freqs).T.astype(np.float32)
    ccat = np.tile(cosT, (4, 1)).astype(ml_dtypes.bfloat16)   # [128, T]
    scat = np.tile(sinT, (4, 1)).astype(ml_dtypes.bfloat16)
    return np.ascontiguousarray(ccat), np.ascontiguousarray(scat)


def _rot_matrix():
    import ml_dtypes
    # rotate_half as a matmul: rot = R @ q (q in [D, T] layout), per 64-row block
    R = np.zeros((D, D), dtype=np.float32)
    R[0:32, 32:64] = -np.eye(32, dtype=np.float32)
    R[32:64, 0:32] = np.eye(32, dtype=np.float32)
    R2 = np.zeros((128, 128), dtype=np.float32)
    R2[0:64, 0:64] = R
    R2[64:128, 64:128] = R
    return np.ascontiguousarray(R2.T.astype(ml_dtypes.bfloat16))  # lhsT for out = R2 @ q


def _build_program():
    import concourse.tile as tile
    from concourse import bacc, mybir

    F32 = mybir.dt.float32
    BF16 = mybir.dt.bfloat16
    EXP = mybir.ActivationFunctionType.Exp

    nc = bacc.Bacc("TRN2", target_bir_lowering=False, debug=False, num_devices=8)

    xT_d = nc.dram_tensor("xT", [C, T], BF16, kind="ExternalInput").ap()
    wq_d = nc.dram_tensor("wq", [C, HPC * D], BF16, kind="ExternalInput").ap()
    wk_d = nc.dram_tensor("wk", [C, HPC * D], BF16, kind="ExternalInput").ap()
    wv_d = nc.dram_tensor("wv", [C, HPC * D], BF16, kind="ExternalInput").ap()
    wo_d = nc.dram_tensor("wo", [HPC * D, C], BF16, kind="ExternalInput").ap()
    ccat_d = nc.dram_tensor("ccat", [128, T], BF16, kind="ExternalInput").ap()
    scat_d = nc.dram_tensor("scat", [128, T], BF16, kind="ExternalInput").ap()
    r2t_d = nc.dram_tensor("r2t", [128, 128], BF16, kind="ExternalInput").ap()
    utri_d = nc.dram_tensor("utri", [128, 128], BF16, kind="ExternalInput").ap()
    eband_d = nc.dram_tensor("eband", [128, 128], BF16, kind="ExternalInput").ap()
    y_d = nc.dram_tensor("y", [T, C], F32, kind="ExternalOutput").ap()

    with tile.TileContext(nc) as tc, ExitStack() as ctx:
        big_pool = ctx.enter_context(tc.tile_pool(name="big", bufs=1))
        q_all = big_pool.tile([128, NP, T], BF16)
        k_all = big_pool.tile([128, NP, T], BF16)
        v_aug = big_pool.tile([128, KC, HPC, D + 1], BF16)
        out_norm = big_pool.tile([128, NP, T], BF16)

        cst_pool = ctx.enter_context(tc.tile_pool(name="cst", bufs=1))
        xt_pool = ctx.enter_context(tc.tile_pool(name="xt", bufs=1))
        xt_sb = xt_pool.tile([128, CC, T], BF16)

        p_sbp = ctx.enter_context(tc.tile_pool(name="p_sb", bufs=12))
        l_sbp = ctx.enter_context(tc.tile_pool(name="l_sb", bufs=3))
        r_sbp = ctx.enter_context(tc.tile_pool(name="r_sb", bufs=3))
        y_sbp = ctx.enter_context(tc.tile_pool(name="y_sb", bufs=2))

        r2t = cst_pool.tile([128, 128], BF16)
        nc.sync.dma_start(r2t[:], r2t_d)
        wv_sb = cst_pool.tile([128, CC, HPC * D], BF16)
        wo_sb = cst_pool.tile([128, NP, C], BF16)
        utri = cst_pool.tile([128, 128], BF16)
        eband = cst_pool.tile([128, 128], BF16)
        exp_warm = cst_pool.tile([1, 2], F32)

        nc.gpsimd.memset(v_aug[:, :, :, D:D + 1], 1.0)

        # ---------- attention building blocks ----------
        def emit_scores_offdiag(p, qc, kcs):
            s_t = [s_psp.tile([128, 1024], F32, tag=f"s{h01}",
                              name=f"s_t{h01}") for h01 in (0, 1)]
            for j, kc in enumerate(kcs):
                for h01 in (0, 1):
                    r0, r1 = h01 * 64, h01 * 64 + 64
                    nc.tensor.matmul(
                        s_t[h01][:, j * 512:(j + 1) * 512],
                        k_all[r0:r1, p, kc * 128:(kc + 1) * 128],
                        q_all[r0:r1, p, qc * 512:(qc + 1) * 512],
                        start=True, stop=True,
                    )
            pts = []
            for h01 in (0, 1):
                pt = p_sbp.tile([128, 1024], BF16, tag=f"pt{h01}")
                w = len(kcs) * 512
                nc.scalar.activation(pt[:, 0:w], s_t[h01][:, 0:w], EXP,
                                     scale=0.125)
                pts.append(pt)
            return pts

        def emit_pv_offdiag(p, qc, kcs, pts, pv):
            for j, kc in enumerate(kcs):
                for h01 in (0, 1):
                    nc.tensor.matmul(
                        pv[h01][:],
                        v_aug[:, kc, p * 2 + h01, :],
                        pts[h01][:, j * 512:(j + 1) * 512],
                        start=(kc == 0), stop=False,
                    )

        # diagonal tiles: half 0 = j0(512)+j1(384), half 1 = j2(256)+j3(128)
        DIAG_SEGS = (((0, 0, 512), (1, 512, 384)),
                     ((2, 0, 256), (3, 256, 128)))

        def emit_scores_diag(p, qc, segs):
            s_d = [s_psp.tile([128, 1024], F32, tag=f"s{h01}",
                              name=f"s_d{h01}") for h01 in (0, 1)]
            for j, off, wj in segs:
                kc = 4 * qc + j
                for h01 in (0, 1):
                    r0, r1 = h01 * 64, h01 * 64 + 64
                    nc.tensor.matmul(
                        s_d[h01][:, off:off + wj],
                        k_all[r0:r1, p, kc * 128:(kc + 1) * 128],
                        q_all[r0:r1, p, qc * 512 + 128 * j:qc * 512 + 512],
                        start=True, stop=False,
                    )
                for h01 in (0, 1):
                    nc.tensor.matmul(
                        s_d[h01][:, off:off + 128],
                        utri[:], eband[:],
                        start=False, stop=True,
                    )
            pts = []
            for h01 in (0, 1):
                pt_d = p_sbp.tile([128, 1024], BF16, tag=f"pt{h01}",
                                  name="pt_d")
                wtot = sum(sg[2] for sg in segs)
                nc.scalar.activation(pt_d[:, 0:wtot], s_d[h01][:, 0:wtot],
                                     EXP, scale=0.125)
                pts.append(pt_d)
            return pts

        def emit_pv_diag(p, qc, segs, pts, pv, last):
            for j, off, wj in segs:
                kc = 4 * qc + j
                for h01 in (0, 1):
                    nc.tensor.matmul(
                        pv[h01][:, 128 * j:512],
                        v_aug[:, kc, p * 2 + h01, :],
                        pts[h01][:, off:off + wj],
                        start=(kc == 0), stop=(last and j == 3),
                    )

        def emit_norm(p, qc, pv, tail=False):
            for h01 in (0, 1):
                lrow = l_sbp.tile([1, 512], F32, tag=f"l{h01}")
                if tail:
                    nc.scalar.copy(lrow[0:1, :], pv[h01][64:65, :])
                else:
                    nc.vector.tensor_copy(lrow[0:1, :], pv[h01][64:65, :])
                rbc = r_sbp.tile([64, 512], F32, tag=f"r{h01}")
                nc.gpsimd.partition_broadcast(rbc[:], lrow[0:1, :],
                                              channels=64)
                nc.vector.reciprocal_approx_fast(rbc[:], rbc[:])
                nc.vector.tensor_mul(
                    out_norm[h01 * 64:h01 * 64 + 64, p,
                             qc * 512:(qc + 1) * 512],
                    pv[h01][0:64, :],
                    rbc[:],
                )

        def attn_units(p, qc):
            units = []
            for g0 in range(0, 4 * qc, 2):
                kcs = list(range(g0, min(g0 + 2, 4 * qc)))
                units.append((
                    (lambda kk: lambda: emit_scores_offdiag(p, qc, kk))(kcs),
                    (lambda kk: lambda pts, pv: emit_pv_offdiag(
                        p, qc, kk, pts, pv))(kcs),
                ))
            for half, segs in enumerate(DIAG_SEGS):
                units.append((
                    (lambda ss: lambda: emit_scores_diag(p, qc, ss))(segs),
                    (lambda ss, la: lambda pts, pv: emit_pv_diag(
                        p, qc, ss, pts, pv, last=la))(segs, half == 1),
                ))
            return units

        # ---- phase 1 + leading (p0, qc3) scores ----
        with tc.tile_pool(name="w", bufs=1) as w_pool, \
             tc.tile_pool(name="const", bufs=1) as const_pool, \
             tc.tile_pool(name="p1ps", bufs=4, space="PSUM") as p1ps, \
             tc.tile_pool(name="p1tmp", bufs=2) as p1tmp:
            wq_sb = w_pool.tile([128, CC, HPC * D], BF16)
            nc.sync.dma_start(wq_sb[:], wq_d.rearrange("(cc p) d -> p cc d", p=128))
            wk_sb = w_pool.tile([128, CC, HPC * D], BF16)
            ccat = const_pool.tile([128, T], BF16)
            scat = const_pool.tile([128, T], BF16)
            nc.scalar.dma_start(wk_sb[:], wk_d.rearrange("(cc p) d -> p cc d", p=128))
            xT_r = xT_d.rearrange("(cc p) t -> p cc t", p=128)
            x_engs = (nc.sync, nc.scalar, nc.gpsimd, nc.sync, nc.scalar,
                      nc.gpsimd)
            for cc in range(CC):
                x_engs[cc].dma_start(xt_sb[:, cc, :], xT_r[:, cc, :])
            nc.sync.dma_start(ccat[:], ccat_d)
            nc.scalar.dma_start(scat[:], scat_d)
            nc.sync.dma_start(utri[:], utri_d)
            nc.scalar.dma_start(eband[:], eband_d)
            nc.gpsimd.dma_start(wv_sb[:], wv_d.rearrange("(cc p) d -> p cc d", p=128))
            nc.gpsimd.dma_start(wo_sb[:], wo_d.rearrange("(hc p) c -> p hc c", p=128))

            # HAM warmup + Exp table preload while the input DMAs land
            warm_t = p1ps.tile([128, 1024], F32, tag="p1")
            warm = warm_t[:, 0:128]
            nc.scalar.activation(exp_warm[:], r2t[0:1, 0:2], EXP)
            for _ in range(44):
                nc.tensor.matmul(warm[:], r2t[:], r2t[:], start=True, stop=True)

            def finish_block(blk):
                dt, dst, qraw, sin_t = blk
                for hh in range(2):
                    hsl = slice(hh * 1024, (hh + 1) * 1024)
                    ps_r = p1ps.tile([128, 1024], F32, tag="p1", name="ps_r")
                    for tq in range(2):
                        nc.tensor.matmul(
                            ps_r[:, tq * 512:(tq + 1) * 512],
                            r2t[:],
                            qraw[:, hh * 1024 + tq * 512:
                                  hh * 1024 + (tq + 1) * 512],
                            start=True, stop=True,
                        )
                    nc.vector.tensor_mul(sin_t[:, hsl], ps_r[:, :], scat[:, hsl])
                nc.vector.tensor_mul(dst[:, dt, :], qraw[:], ccat[:])
                nc.vector.tensor_add(dst[:, dt, :], dst[:, dt, :], sin_t[:])

            prev = None
            for dt in range(NP):
                for w_sb, dst in ((wq_sb, q_all), (wk_sb, k_all)):
                    qraw = p1tmp.tile([128, T], BF16, tag="qraw")
                    sin_t = p1tmp.tile([128, T], BF16, tag="sin")
                    for hh in range(2):
                        hsl = slice(hh * 1024, (hh + 1) * 1024)
                        ps_q = p1ps.tile([128, 1024], F32, tag="p1", name="ps_q")
                        for cc in range(CC):
                            for tq in range(2):
                                nc.tensor.matmul(
                                    ps_q[:, tq * 512:(tq + 1) * 512],
                                    w_sb[:, cc, dt * 128:(dt + 1) * 128],
                                    xt_sb[:, cc,
                                          hh * 1024 + tq * 512:
                                          hh * 1024 + (tq + 1) * 512],
                                    start=(cc == 0), stop=(cc == CC - 1),
                                )
                        nc.scalar.copy(qraw[:, hsl], ps_q[:, :])
                    if prev is not None:
                        finish_block(prev)
                    prev = (dt, dst, qraw, sin_t)

            finish_block(prev)

        # ---- phase 2: attention; vproj zipped into (p0, qc3) ----
        with tc.tile_pool(name="s_ps", bufs=1, space="PSUM") as s_psp, \
             tc.tile_pool(name="aux_ps", bufs=4, space="PSUM") as aux_psp:

            def emit_vproj(tt):
                ps_v = aux_psp.tile([128, HPC * D], F32, tag="aux", name="ps_v")
                for cc in range(CC):
                    nc.tensor.matmul(
                        ps_v[:, 0:HPC * D],
                        xt_sb[:, cc, tt * 128:(tt + 1) * 128],
                        wv_sb[:, cc, :],
                        start=(cc == 0), stop=(cc == CC - 1),
                    )
                nc.vector.tensor_copy(
                    v_aug[:, tt, :, 0:D],
                    ps_v[:, 0:HPC * D].rearrange("p (h d) -> p h d", d=D),
                )

            def emit_outproj(qc, tail=False):
                for tt in range(4 * qc, 4 * qc + 4):
                    y_a = aux_psp.tile([128, 512], F32, tag="aux", name="y_a")
                    y_b = aux_psp.tile([128, 256], F32, tag="aux", name="y_b")
                    for hc in range(NP):
                        lhsT = out_norm[:, hc, tt * 128:(tt + 1) * 128]
                        nc.tensor.matmul(y_a[:, 0:512], lhsT,
                                         wo_sb[:, hc, 0:512],
                                         start=(hc == 0), stop=(hc == NP - 1))
                        nc.tensor.matmul(y_b[:, 0:256], lhsT,
                                         wo_sb[:, hc, 512:768],
                                         start=(hc == 0), stop=(hc == NP - 1))
                    yt = y_sbp.tile([128, C], F32, tag="yt")
                    if tail:
                        nc.scalar.copy(yt[:, 0:512], y_a[:, 0:512])
                        nc.scalar.copy(yt[:, 512:768], y_b[:, 0:256])
                    else:
                        nc.vector.tensor_copy(yt[:, 0:512], y_a[:, 0:512])
                        nc.vector.tensor_copy(yt[:, 512:768], y_b[:, 0:256])
                    nc.sync.dma_start(y_d[tt * 128:(tt + 1) * 128, :], yt[:])

            def emit_attn(p, qc, pv, fillers=None, units=None, pre=None):
                """Ping-pong with 1-group score lookahead; optional PE filler
                work (e.g. vproj closures) interleaved between groups."""
                fillers = list(fillers or [])
                fi = 0
                queue = list(pre or [])   # [(pv_fn, pts)] already scored
                for si, (sc_fn, pv_fn) in enumerate(units if units is not None
                                                    else attn_units(p, qc)):
                    queue.append((pv_fn, sc_fn()))
                    if len(queue) > 1:
                        fn, pts = queue.pop(0)
                        fn(pts, pv)
                    while fi < len(fillers) and fi < (si + 1) * 2:
                        fillers[fi]()
                        fi += 1
                while fi < len(fillers):
                    fillers[fi]()
                    fi += 1
                for fn, pts in queue:
                    fn(pts, pv)
                emit_norm(p, qc, pv)

            # qc=3: p0 continues from the phase-1 prefetched groups, with
            # the v-projection zipped in as PE filler
            vproj_fillers = [(lambda t: lambda: emit_vproj(t))(tt)
                             for tt in range(KC)]
            pv = [aux_psp.tile([65, 512], F32, tag="aux",
                               name=f"pv{h01}") for h01 in (0, 1)]
            emit_attn(0, 3, pv, fillers=vproj_fillers)
            for p in (1, 2):
                pv = [aux_psp.tile([65, 512], F32, tag="aux",
                                   name=f"pv{h01}") for h01 in (0, 1)]
                emit_attn(p, 3, pv)
            emit_outproj(3)
            for p in range(NP):
                pv = [aux_psp.tile([65, 512], F32, tag="aux",
                                   name=f"pv{h01}") for h01 in (0, 1)]
                emit_attn(p, 2, pv)
            emit_outproj(2)

            # qc=1 and qc=0 tails: breadth-first across head-pairs to
            # overlap the short scores/exp/PV/norm latency chains
            for qc in (1, 0):
                helds = []
                for p in range(NP):
                    helds.append([(pv_fn, sc_fn())
                                  for sc_fn, pv_fn in attn_units(p, qc)])
                for p in range(NP):
                    pv = [aux_psp.tile([65, 512], F32, tag="aux",
                                       name=f"pv{h01}") for h01 in (0, 1)]
                    for pv_fn, pts in helds[p]:
                        pv_fn(pts, pv)
                    emit_norm(p, qc, pv, tail=True)
                emit_outproj(qc, tail=(qc == 0))

    nc.compile()
    return nc


# make mybir importable inside _build_program's nested scopes
from concourse import mybir  # noqa: E402


def _get_compiled():
    global _COMPILED
    if _COMPILED is None:
        _COMPILED = _build_program()
    return _COMPILED


def _make_in_maps(inputs):
    import ml_dtypes

    BF = ml_dtypes.bfloat16
    x = np.asarray(inputs["x"], dtype=np.float32)
    wq = np.asarray(inputs["wq"], dtype=np.float32).astype(BF)
    wk = np.asarray(inputs["wk"], dtype=np.float32).astype(BF)
    wv = np.asarray(inputs["wv"], dtype=np.float32).astype(BF)
    wo = np.asarray(inputs["wo"], dtype=np.float32).astype(BF)

    ccat, scat = _rope_tables()
    r2t = _rot_matrix()
    m = np.arange(128)
    utri = (m[:, None] <= m[None, :]).astype(BF)
    eband = np.zeros((128, 128), dtype=np.float32)
    eband[np.arange(1, 128), np.arange(127)] = -1e9
    eband = eband.astype(BF)

    xTs = [np.ascontiguousarray(x[b].T.astype(BF)) for b in range(B)]
    in_maps = []
    for c in range(8):
        b, g = c // 2, c % 2
        sl = slice(g * HPC * D, (g + 1) * HPC * D)
        in_maps.append(dict(
            xT=xTs[b],
            wq=np.ascontiguousarray(wq[:, sl]),
            wk=np.ascontiguousarray(wk[:, sl]),
            wv=np.ascontiguousarray(wv[:, sl]),
            wo=np.ascontiguousarray(wo[sl, :]),
            ccat=ccat, scat=scat, r2t=r2t, utri=utri, eband=eband,
        ))
    return in_maps


def kernel(x, wq, wk, wv, wo, mask):
    """Full inputs in, full output out. Shards across 8 NeuronCores internally.

    The mask input is the standard causal mask produced by setup_inputs();
    causality is implemented directly on-device.
    """
    from concourse.bass_utils import run_bass_kernel_spmd

    in_maps = _make_in_maps(dict(x=x, wq=wq, wk=wk, wv=wv, wo=wo))

    nc = _get_compiled()
    res = run_bass_kernel_spmd(nc, in_maps, list(range(8)))
    out = np.empty((B, T, C), dtype=np.float32)
    for b in range(B):
        out[b] = res.results[2 * b]["y"] + res.results[2 * b + 1]["y"]
    return out


# revision 22
# speedup vs baseline: 1.3324x; 1.0078x over previous
"""Causal multi-head attention with RoPE for Trainium2, sharded over 8 NeuronCores.

Problem: B=4, T=2048, C=768, H=12, D=64, fp32 in/out.
    q,k,v = x @ wq/wk/wv  (per-head reshape), RoPE(q,k), causal softmax(q k^T/sqrt(D)) v,
    out = concat_heads @ wo.

Sharding: core c -> (batch b = c//2, head-group g = c%2 covering heads g*6..g*6+5).
Each core computes its 6 heads' attention and a partial output projection
y_c = out_heads(g) @ wo[rows g]; the host sums the two partials per batch.

On-core dataflow (bf16 matmul operands, fp32 PSUM accumulation; bf16 enables
Fast Weight Load and 1024-wide moving operands, so the q/k projection and
rotation chains use half the matmul instructions):
  - host passes x^T (bf16); input DMAs split across all three DGE queues,
    critical tensors first; a HAM-warmup matmul stream covers the load.
  - q^T,k^T in [head_dim, T] layout; RoPE via a block-rotation matmul +
    cos/sin tensor ops on DVE, pipelined one block behind the projection
    so the PSUM->SBUF copy latency never stalls the PE.
  - scores transposed: S^T[k, q] = k^T.T @ q^T with K=64 row-pairing
    (even head at partitions 0:64, odd at 64:128 -> concurrent row groups).
    The first two (p=0, qc=3) score groups are emitted inside the phase-1
    tail and the rest zip against the v-projection, so the ScalarE exp
    stream (the second serial resource, ~13.4M exps/core) starts the
    moment phase 1 drains.
  - P = exp(S/8) on ScalarE -> bf16; causal masking of diagonal tiles via
    a bf16 utri@eband matmul add before exp.
  - PV with a ones-row appended to V: out_unnorm^T[d, q] and l[q] in one
    accumulated matmul chain per (head, q-chunk); 1-group score lookahead
    keeps the PE ahead of the exp stream.
  - normalization: l row -> partition 0, gpsimd.partition_broadcast,
    reciprocal_approx_fast, TT multiply.  The small qc=1/qc=0 tails run
    breadth-first across head-pairs with their PSUM->SBUF copies moved to
    the by-then idle ScalarE.
  - output projection accumulates 3 head-pair chunks into [128, 768] PSUM.
"""

import numpy as np
from contextlib import ExitStack

B, T, C, H, D = 4, 2048, 768, 12, 64
HPC = 6          # heads per core
NP = 3           # head-pair tiles per core
CC = C // 128    # 6 contraction chunks
TT = T // 128    # 16 t tiles
QC = T // 512    # 4 q chunks
KC = T // 128    # 16 k chunks

_COMPILED = None


def _rope_tables():
    import ml_dtypes
    inv_freq = 1.0 / (10000.0 ** (np.arange(0, D, 2, dtype=np.float64) / D))  # [32]
    t = np.arange(T, dtype=np.float64)
    freqs = np.outer(t, inv_freq)                      # [T, 32]
    cosT = np.cos(freqs).T.astype(np.float32)          # [32, T]
    sinT = np.sin(freqs).T.astype(np.float32)
    ccat = np.tile(cosT, (4, 1)).astype(ml_dtypes.bfloat16)   # [128, T]
    scat = np.tile(sinT, (4, 1)).astype(ml_dtypes.bfloat16)
    return np.ascontiguousarray(ccat), np.ascontiguousarray(scat)


def _rot_matrix():
    import ml_dtypes
    # rotate_half as a matmul: rot = R @ q (q in [D, T] layout), per 64-row block
    R = np.zeros((D, D), dtype=np.float32)
    R[0:32, 32:64] = -np.eye(32, dtype=np.float32)
    R[32:64, 0:32] = np.eye(32, dtype=np.float32)
    R2 = np.zeros((128, 128), dtype=np.float32)
    R2[0:64, 0:64] = R
    R2[64:128, 64:128] = R
    return np.ascontiguousarray(R2.T.astype(ml_dtypes.bfloat16))  # lhsT for out = R2 @ q


def _build_program():
    import concourse.tile as tile
    from concourse import bacc, mybir

    F32 = mybir.dt.float32
    BF16 = mybir.dt.bfloat16
    EXP = mybir.ActivationFunctionType.Exp

    nc = bacc.Bacc("TRN2", target_bir_lowering=False, debug=False, num_devices=8)

    xT_d = nc.dram_tensor("xT", [C, T], BF16, kind="ExternalInput").ap()
    wq_d = nc.dram_tensor("wq", [C, HPC * D], BF16, kind="ExternalInput").ap()
    wk_d = nc.dram_tensor("wk", [C, HPC * D], BF16, kind="ExternalInput").ap()
    wv_d = nc.dram_tensor("wv", [C, HPC * D], BF16, kind="ExternalInput").ap()
    wo_d = nc.dram_tensor("wo", [HPC * D, C], BF16, kind="ExternalInput").ap()
    ccat_d = nc.dram_tensor("ccat", [128, T], BF16, kind="ExternalInput").ap()
    scat_d = nc.dram_tensor("scat", [128, T], BF16, kind="ExternalInput").ap()
    r2t_d = nc.dram_tensor("r2t", [128, 128], BF16, kind="ExternalInput").ap()
    utri_d = nc.dram_tensor("utri", [128, 128], BF16, kind="ExternalInput").ap()
    eband_d = nc.dram_tensor("eband", [128, 128], BF16, kind="ExternalInput").ap()
    y_d = nc.dram_tensor("y", [T, C], F32, kind="ExternalOutput").ap()

    with tile.TileContext(nc) as tc, ExitStack() as ctx:
        big_pool = ctx.enter_context(tc.tile_pool(name="big", bufs=1))
        q_all = big_pool.tile([128, NP, T], BF16)
        k_all = big_pool.tile([128, NP, T], BF16)
        v_aug = big_pool.tile([128, KC, HPC, D + 1], BF16)
        out_norm = big_pool.tile([128, NP, T], BF16)

        cst_pool = ctx.enter_context(tc.tile_pool(name="cst", bufs=1))
        xt_pool = ctx.enter_context(tc.tile_pool(name="xt", bufs=1))
        xt_sb = xt_pool.tile([128, CC, T], BF16)

        p_sbp = ctx.enter_context(tc.tile_pool(name="p_sb", bufs=12))
        l_sbp = ctx.enter_context(tc.tile_pool(name="l_sb", bufs=3))
        r_sbp = ctx.enter_context(tc.tile_pool(name="r_sb", bufs=3))
        y_sbp = ctx.enter_context(tc.tile_pool(name="y_sb", bufs=2))

        r2t = cst_pool.tile([128, 128], BF16)
        nc.sync.dma_start(r2t[:], r2t_d)
        wv_sb = cst_pool.tile([128, CC, HPC * D], BF16)
        wo_sb = cst_pool.tile([128, NP, C], BF16)
        utri = cst_pool.tile([128, 128], BF16)
        eband = cst_pool.tile([128, 128], BF16)
        exp_warm = cst_pool.tile([1, 2], F32)

        nc.gpsimd.memset(v_aug[:, :, :, D:D + 1], 1.0)

        # ---------- attention building blocks ----------
        def emit_scores_offdiag(p, qc, kcs):
            s_t = [s_psp.tile([128, 1024], F32, tag=f"s{h01}",
                              name=f"s_t{h01}") for h01 in (0, 1)]
            for j, kc in enumerate(kcs):
                for h01 in (0, 1):
                    r0, r1 = h01 * 64, h01 * 64 + 64
                    nc.tensor.matmul(
                        s_t[h01][:, j * 512:(j + 1) * 512],
                        k_all[r0:r1, p, kc * 128:(kc + 1) * 128],
                        q_all[r0:r1, p, qc * 512:(qc + 1) * 512],
                        start=True, stop=True,
                    )
            pts = []
            for h01 in (0, 1):
                pt = p_sbp.tile([128, 1024], BF16, tag=f"pt{h01}")
                w = len(kcs) * 512
                nc.scalar.activation(pt[:, 0:w], s_t[h01][:, 0:w], EXP,
                                     scale=0.125)
                pts.append(pt)
            return pts

        def emit_pv_offdiag(p, qc, kcs, pts, pv):
            for j, kc in enumerate(kcs):
                for h01 in (0, 1):
                    nc.tensor.matmul(
                        pv[h01][:],
                        v_aug[:, kc, p * 2 + h01, :],
                        pts[h01][:, j * 512:(j + 1) * 512],
                        start=(kc == 0), stop=False,
                    )

        # diagonal tiles: half 0 = j0(512)+j1(384), half 1 = j2(256)+j3(128)
        DIAG_SEGS = (((0, 0, 512), (1, 512, 384)),
                     ((2, 0, 256), (3, 256, 128)))

        def emit_scores_diag(p, qc, segs):
            s_d = [s_psp.tile([128, 1024], F32, tag=f"s{h01}",
                              name=f"s_d{h01}") for h01 in (0, 1)]
            for j, off, wj in segs:
                kc = 4 * qc + j
                for h01 in (0, 1):
                    r0, r1 = h01 * 64, h01 * 64 + 64
                    nc.tensor.matmul(
                        s_d[h01][:, off:off + wj],
                        k_all[r0:r1, p, kc * 128:(kc + 1) * 128],
                        q_all[r0:r1, p, qc * 512 + 128 * j:qc * 512 + 512],
                        start=True, stop=False,
                    )
                for h01 in (0, 1):
                    nc.tensor.matmul(
                        s_d[h01][:, off:off + 128],
                        utri[:], eband[:],
                        start=False, stop=True,
                    )
            pts = []
            for h01 in (0, 1):
                pt_d = p_sbp.tile([128, 1024], BF16, tag=f"pt{h01}",
                                  name="pt_d")
                wtot = sum(sg[2] for sg in segs)
                nc.scalar.activation(pt_d[:, 0:wtot], s_d[h01][:, 0:wtot],
                                     EXP, scale=0.125)
                pts.append(pt_d)
            return pts

        def emit_pv_diag(p, qc, segs, pts, pv, last):
            for j, off, wj in segs:
                kc = 4 * qc + j
                for h01 in (0, 1):
                    nc.tensor.matmul(
                        pv[h01][:, 128 * j:512],
                        v_aug[:, kc, p * 2 + h01, :],
                        pts[h01][:, off:off + wj],
                        start=(kc == 0), stop=(last and j == 3),
                    )

        def emit_norm(p, qc, pv, tail=False):
            for h01 in (0, 1):
                lrow = l_sbp.tile([1, 512], F32, tag=f"l{h01}")
                if tail:
                    nc.scalar.copy(lrow[0:1, :], pv[h01][64:65, :])
                else:
                    nc.vector.tensor_copy(lrow[0:1, :], pv[h01][64:65, :])
                rbc = r_sbp.tile([64, 512], F32, tag=f"r{h01}")
                nc.gpsimd.partition_broadcast(rbc[:], lrow[0:1, :],
                                              channels=64)
                nc.vector.reciprocal_approx_fast(rbc[:], rbc[:])
                nc.vector.tensor_mul(
                    out_norm[h01 * 64:h01 * 64 + 64, p,
                             qc * 512:(qc + 1) * 512],
                    pv[h01][0:64, :],
                    rbc[:],
                )

        def attn_units(p, qc):
            units = []
            for g0 in range(0, 4 * qc, 2):
                kcs = list(range(g0, min(g0 + 2, 4 * qc)))
                units.append((
                    (lambda kk: lambda: emit_scores_offdiag(p, qc, kk))(kcs),
                    (lambda kk: lambda pts, pv: emit_pv_offdiag(
                        p, qc, kk, pts, pv))(kcs),
                ))
            for half, segs in enumerate(DIAG_SEGS):
                units.append((
                    (lambda ss: lambda: emit_scores_diag(p, qc, ss))(segs),
                    (lambda ss, la: lambda pts, pv: emit_pv_diag(
                        p, qc, ss, pts, pv, last=la))(segs, half == 1),
                ))
            return units

        # ---- phase 1 + leading (p0, qc3) scores ----
        with tc.tile_pool(name="w", bufs=1) as w_pool, \
             tc.tile_pool(name="const", bufs=1) as const_pool, \
             tc.tile_pool(name="p1ps", bufs=4, space="PSUM") as p1ps, \
             tc.tile_pool(name="p1tmp", bufs=2) as p1tmp:
            wq_sb = w_pool.tile([128, CC, HPC * D], BF16)
            nc.sync.dma_start(wq_sb[:], wq_d.rearrange("(cc p) d -> p cc d", p=128))
            wk_sb = w_pool.tile([128, CC, HPC * D], BF16)
            ccat = const_pool.tile([128, T], BF16)
            scat = const_pool.tile([128, T], BF16)
            nc.scalar.dma_start(wk_sb[:], wk_d.rearrange("(cc p) d -> p cc d", p=128))
            xT_r = xT_d.rearrange("(cc p) t -> p cc t", p=128)
            x_engs = (nc.sync, nc.scalar, nc.gpsimd, nc.sync, nc.scalar,
                      nc.gpsimd)
            for cc in range(CC):
                x_engs[cc].dma_start(xt_sb[:, cc, :], xT_r[:, cc, :])
            nc.sync.dma_start(ccat[:], ccat_d)
            nc.scalar.dma_start(scat[:], scat_d)
            nc.sync.dma_start(utri[:], utri_d)
            nc.scalar.dma_start(eband[:], eband_d)
            nc.gpsimd.dma_start(wv_sb[:], wv_d.rearrange("(cc p) d -> p cc d", p=128))
            nc.gpsimd.dma_start(wo_sb[:], wo_d.rearrange("(hc p) c -> p hc c", p=128))

            # HAM warmup + Exp table preload while the input DMAs land
            warm_t = p1ps.tile([128, 1024], F32, tag="p1")
            warm = warm_t[:, 0:128]
            nc.scalar.activation(exp_warm[:], r2t[0:1, 0:2], EXP)
            for _ in range(44):
                nc.tensor.matmul(warm[:], r2t[:], r2t[:], start=True, stop=True)

            def finish_block(blk):
                dt, dst, qraw, sin_t = blk
                for hh in range(2):
                    hsl = slice(hh * 1024, (hh + 1) * 1024)
                    ps_r = p1ps.tile([128, 1024], F32, tag="p1", name="ps_r")
                    for tq in range(2):
                        nc.tensor.matmul(
                            ps_r[:, tq * 512:(tq + 1) * 512],
                            r2t[:],
                            qraw[:, hh * 1024 + tq * 512:
                                  hh * 1024 + (tq + 1) * 512],
                            start=True, stop=True,
                        )
                    nc.vector.tensor_mul(sin_t[:, hsl], ps_r[:, :], scat[:, hsl])
                nc.vector.tensor_mul(dst[:, dt, :], qraw[:], ccat[:])
                nc.vector.tensor_add(dst[:, dt, :], dst[:, dt, :], sin_t[:])

            blocks = [(dt, w_sb, dst)
                      for dt in range(NP)
                      for w_sb, dst in ((wq_sb, q_all), (wk_sb, k_all))]

            # The first two blocks run cc-interleaved: four PSUM chains
            # consume each x chunk as its DMA lands (the load is HBM-bound,
            # ~3us/chunk), instead of one chain starving on later chunks.
            lead, lead_ps = [], []
            for dt, w_sb, dst in blocks[:2]:
                qraw = p1tmp.tile([128, T], BF16, tag="qraw")
                sin_t = p1tmp.tile([128, T], BF16, tag="sin")
                ps_pair = [p1ps.tile([128, 1024], F32, tag="p1", name="ps_q")
                           for _ in range(2)]
                lead.append((dt, w_sb, dst, qraw, sin_t))
                lead_ps.append(ps_pair)
            for cc in range(CC):
                for bi, (dt, w_sb, dst, qraw, sin_t) in enumerate(lead):
                    for hh in range(2):
                        for tq in range(2):
                            nc.tensor.matmul(
                                lead_ps[bi][hh][:, tq * 512:(tq + 1) * 512],
                                w_sb[:, cc, dt * 128:(dt + 1) * 128],
                                xt_sb[:, cc,
                                      hh * 1024 + tq * 512:
                                      hh * 1024 + (tq + 1) * 512],
                                start=(cc == 0), stop=(cc == CC - 1),
                            )
            for bi, (dt, w_sb, dst, qraw, sin_t) in enumerate(lead):
                for hh in range(2):
                    hsl = slice(hh * 1024, (hh + 1) * 1024)
                    nc.scalar.copy(qraw[:, hsl], lead_ps[bi][hh][:, :])

            prev = (lead[1][0], lead[1][2], lead[1][3], lead[1][4])
            finish_block((lead[0][0], lead[0][2], lead[0][3], lead[0][4]))
            for dt, w_sb, dst in blocks[2:]:
                qraw = p1tmp.tile([128, T], BF16, tag="qraw")
                sin_t = p1tmp.tile([128, T], BF16, tag="sin")
                for hh in range(2):
                    hsl = slice(hh * 1024, (hh + 1) * 1024)
                    ps_q = p1ps.tile([128, 1024], F32, tag="p1", name="ps_q")
                    for cc in range(CC):
                        for tq in range(2):
                            nc.tensor.matmul(
                                ps_q[:, tq * 512:(tq + 1) * 512],
                                w_sb[:, cc, dt * 128:(dt + 1) * 128],
                                xt_sb[:, cc,
                                      hh * 1024 + tq * 512:
                                      hh * 1024 + (tq + 1) * 512],
                                start=(cc == 0), stop=(cc == CC - 1),
                            )
                    nc.scalar.copy(qraw[:, hsl], ps_q[:, :])
                finish_block(prev)
                prev = (dt, dst, qraw, sin_t)

            finish_block(prev)

        # ---- phase 2: attention; vproj zipped into (p0, qc3) ----
        with tc.tile_pool(name="s_ps", bufs=1, space="PSUM") as s_psp, \
             tc.tile_pool(name="aux_ps", bufs=4, space="PSUM") as aux_psp:

            def emit_vproj(tt):
                ps_v = aux_psp.tile([128, HPC * D], F32, tag="aux", name="ps_v")
                for cc in range(CC):
                    nc.tensor.matmul(
                        ps_v[:, 0:HPC * D],
                        xt_sb[:, cc, tt * 128:(tt + 1) * 128],
                        wv_sb[:, cc, :],
                        start=(cc == 0), stop=(cc == CC - 1),
                    )
                nc.vector.tensor_copy(
                    v_aug[:, tt, :, 0:D],
                    ps_v[:, 0:HPC * D].rearrange("p (h d) -> p h d", d=D),
                )

            def emit_outproj(qc, tail=False):
                for tt in range(4 * qc, 4 * qc + 4):
                    y_a = aux_psp.tile([128, 512], F32, tag="aux", name="y_a")
                    y_b = aux_psp.tile([128, 256], F32, tag="aux", name="y_b")
                    for hc in range(NP):
                        lhsT = out_norm[:, hc, tt * 128:(tt + 1) * 128]
                        nc.tensor.matmul(y_a[:, 0:512], lhsT,
                                         wo_sb[:, hc, 0:512],
                                         start=(hc == 0), stop=(hc == NP - 1))
                        nc.tensor.matmul(y_b[:, 0:256], lhsT,
                                         wo_sb[:, hc, 512:768],
                                         start=(hc == 0), stop=(hc == NP - 1))
                    yt = y_sbp.tile([128, C], F32, tag="yt")
                    if tail:
                        nc.scalar.copy(yt[:, 0:512], y_a[:, 0:512])
                        nc.scalar.copy(yt[:, 512:768], y_b[:, 0:256])
                    else:
                        nc.vector.tensor_copy(yt[:, 0:512], y_a[:, 0:512])
                        nc.vector.tensor_copy(yt[:, 512:768], y_b[:, 0:256])
                    nc.sync.dma_start(y_d[tt * 128:(tt + 1) * 128, :], yt[:])

            def emit_attn(p, qc, pv, fillers=None, units=None, pre=None):
                """Ping-pong with 1-group score lookahead; optional PE filler
                work (e.g. vproj closures) interleaved between groups."""
                fillers = list(fillers or [])
                fi = 0
                queue = list(pre or [])   # [(pv_fn, pts)] already scored
                for si, (sc_fn, pv_fn) in enumerate(units if units is not None
                                                    else attn_units(p, qc)):
                    queue.append((pv_fn, sc_fn()))
                    if len(queue) > 1:
                        fn, pts = queue.pop(0)
                        fn(pts, pv)
                    while fi < len(fillers) and fi < (si + 1) * 2:
                        fillers[fi]()
                        fi += 1
                while fi < len(fillers):
                    fillers[fi]()
                    fi += 1
                for fn, pts in queue:
                    fn(pts, pv)
                emit_norm(p, qc, pv)

            # qc=3: p0 continues from the phase-1 prefetched groups, with
            # the v-projection zipped in as PE filler
            vproj_fillers = [(lambda t: lambda: emit_vproj(t))(tt)
                             for tt in range(KC)]
            pv = [aux_psp.tile([65, 512], F32, tag="aux",
                               name=f"pv{h01}") for h01 in (0, 1)]
            emit_attn(0, 3, pv, fillers=vproj_fillers)
            for p in (1, 2):
                pv = [aux_psp.tile([65, 512], F32, tag="aux",
                                   name=f"pv{h01}") for h01 in (0, 1)]
                emit_attn(p, 3, pv)
            emit_outproj(3)
            for p in range(NP):
                pv = [aux_psp.tile([65, 512], F32, tag="aux",
                                   name=f"pv{h01}") for h01 in (0, 1)]
                emit_attn(p, 2, pv)
            emit_outproj(2)

            # qc=1 and qc=0 tails: breadth-first across head-pairs to
            # overlap the short scores/exp/PV/norm latency chains; qc=0's
            # scores/exps run under qc=1's PV phase and outproj(1) fills
            # the PE during qc=0's exps
            helds1 = [[(pv_fn, sc_fn()) for sc_fn, pv_fn in attn_units(p, 1)]
                      for p in range(NP)]
            for p in range(NP):
                pv = [aux_psp.tile([65, 512], F32, tag="aux",
                                   name=f"pv{h01}") for h01 in (0, 1)]
                for pv_fn, pts in helds1[p]:
                    pv_fn(pts, pv)
                emit_norm(p, 1, pv, tail=True)
            helds0 = [[(pv_fn, sc_fn()) for sc_fn, pv_fn in attn_units(p, 0)]
                      for p in range(NP)]
            emit_outproj(1)
            for p in range(NP):
                pv = [aux_psp.tile([65, 512], F32, tag="aux",
                                   name=f"pv{h01}") for h01 in (0, 1)]
                for pv_fn, pts in helds0[p]:
                    pv_fn(pts, pv)
                emit_norm(p, 0, pv, tail=True)
            emit_outproj(0, tail=True)

    nc.compile()
    return nc


# make mybir importable inside _build_program's nested scopes
from concourse import mybir  # noqa: E402


def _get_compiled():
    global _COMPILED
    if _COMPILED is None:
        _COMPILED = _build_program()
    return _COMPILED


def _make_in_maps(inputs):
    import ml_dtypes

    BF = ml_dtypes.bfloat16
    x = np.asarray(inputs["x"], dtype=np.float32)
    wq = np.asarray(inputs["wq"], dtype=np.float32).astype(BF)
    wk = np.asarray(inputs["wk"], dtype=np.float32).astype(BF)
    wv = np.asarray(inputs["wv"], dtype=np.float32).astype(BF)
    wo = np.asarray(inputs["wo"], dtype=np.float32).astype(BF)

    ccat, scat = _rope_tables()
    r2t = _rot_matrix()
    m = np.arange(128)
    utri = (m[:, None] <= m[None, :]).astype(BF)
    eband = np.zeros((128, 128), dtype=np.float32)
    eband[np.arange(1, 128), np.arange(127)] = -1e9
    eband = eband.astype(BF)

    xTs = [np.ascontiguousarray(x[b].T.astype(BF)) for b in range(B)]
    in_maps = []
    for c in range(8):
        b, g = c // 2, c % 2
        sl = slice(g * HPC * D, (g + 1) * HPC * D)
        in_maps.append(dict(
            xT=xTs[b],
            wq=np.ascontiguousarray(wq[:, sl]),
            wk=np.ascontiguousarray(wk[:, sl]),
            wv=np.ascontiguousarray(wv[:, sl]),
            wo=np.ascontiguousarray(wo[sl, :]),
            ccat=ccat, scat=scat, r2t=r2t, utri=utri, eband=eband,
        ))
    return in_maps


def kernel(x, wq, wk, wv, wo, mask):
    """Full inputs in, full output out. Shards across 8 NeuronCores internally.

    The mask input is the standard causal mask produced by setup_inputs();
    causality is implemented directly on-device.
    """
    from concourse.bass_utils import run_bass_kernel_spmd

    in_maps = _make_in_maps(dict(x=x, wq=wq, wk=wk, wv=wv, wo=wo))

    nc = _get_compiled()
    res = run_bass_kernel_spmd(nc, in_maps, list(range(8)))
    out = np.empty((B, T, C), dtype=np.float32)
    for b in range(B):
        out[b] = res.results[2 * b]["y"] + res.results[2 * b + 1]["y"]
    return out
